# revision 7
# baseline (speedup 1.0000x reference)
"""Trainium2 Bass kernel for the Expected-Depth DP loss.

Computation (see reference):
  - edge_max = max over first 7 of 8 op-logits          [S, 64, 16]
  - w        = masked softmax over the 16-wide window   [S, 64, 16]
  - DP scan:  ed[j] = sum_k w[j,k] * (ed[base+k] + 1),  j = 2..65
  - loss     = sum_s theta[s] * softmax(beta[s]) . (ed[ii] + ed[jj])

Sharding: S=8192 stages split across 8 cores (pure data parallel,
1024 stages/core as 128 partitions x 8 free slots). Per-core partial
losses are summed on the host.

v2 layout/engine choices:
  - alpha staged in HBM as 7 op-major fp8(e4m3) planes (op 7 unused),
    node-grouped; SWDGE DMAs cast fp8 -> bf16 inline, so HBM alpha
    traffic is 1/4 of f32 while SBUF/DVE stay bf16.
  - max-of-7 as a 4-instruction bf16 tensor_tensor max tree (2x mode)
    instead of a 1x tensor_reduce.
  - per-node-group pipeline: tree/exp/softmax/DP for nodes [16g,16g+16)
    overlap the next group's plane DMA.
  - the softmax reciprocal is broadcast-expanded on the scalar engine so
    the normalize multiply runs dense bf16 at DVE 2x.
  - beta rides the sync-engine HWDGE ring in bf16; its exps/matmuls are
    emitted after the group loop so they only fill scalar-engine gaps.
"""

import numpy as np

SW = 16          # DP window
NN = 64          # nodes per stage
S = 8192         # stages
E = 2016         # beta edges
P = 128          # SBUF partitions
N_CORES = 8
S_CORE = S // N_CORES        # 1024
T = S_CORE // P              # 8 stage slots per partition
NG = 4                       # node groups
GN = NN // NG                # 16 nodes per group
GW = GN * SW                 # 256 edge_max floats per stage per group
GF = T * GW                  # 2048 free elems per group tile
NPL = 7                      # op planes
EDW = 67                     # ed row stride (66 node slots + 1 pad)
NCH = 16                     # beta column chunks
ECH = E // NCH               # 126 edges per chunk
NMASK = 14                   # nodes with partially-valid windows

_CACHE = {}


def _host_consts():
    import ml_dtypes

    ii, jj = [], []
    for i in range(2, NN + 1):
        for j in range(i + 1, NN + 2):
            ii.append(i)
            jj.append(j)
    ii = np.asarray(ii)
    jj = np.asarray(jj)
    # incidence matrix chunks: mt[e_local, c*67 + k] = [ii==k] + [jj==k],
    # column 66 of each chunk is all ones (softmax denominator)
    mt = np.zeros((NCH, ECH, EDW), np.float32)
    for e in range(E):
        c, el = divmod(e, ECH)
        mt[c, el, ii[e]] += 1.0
        mt[c, el, jj[e]] += 1.0
        mt[c, el, EDW - 1] = 1.0
    mt = np.ascontiguousarray(
        mt.transpose(1, 0, 2).reshape(ECH, NCH * EDW)
    ).astype(ml_dtypes.bfloat16)
    # validity mask for the first 14 nodes (node n: rows k < n+2 valid)
    mask = np.zeros((NMASK, SW), np.float32)
    for n in range(NMASK):
        mask[n, : n + 2] = 1.0
    mask = np.ascontiguousarray(
        np.broadcast_to(mask.reshape(1, NMASK * SW), (P, NMASK * SW))
    ).astype(ml_dtypes.bfloat16)
    return mt, mask


def _install_tile_patches():
    import concourse.mybir as mybir
    from concourse.tile import TileContext
    from concourse.vector_clock import ScopedClock, VectorClock

    # This walrus build rejects TPB instructions carrying more than one sem
    # wait (two for EventSemaphore, zero for Pool-engine non-ES ops), but
    # Tile's wait assignment happily packs 2-3. Split the extras onto
    # single-wait NoOps (ES chunks for Pool) on the same engine.
    if not getattr(TileContext, "_ant_wait_split", False):
        _orig_commit = TileContext._commit_instruction

        def _commit_split(self, inst, lazy_reg_writes=True):
            si = inst.sync_info
            is_es = isinstance(inst, mybir.InstEventSemaphore)
            is_pool = inst.engine == mybir.EngineType.Pool
            limit = 2 if is_es else (0 if is_pool else 1)
            if si is not None and si.on_wait and len(si.on_wait) > limit:
                waits = list(si.on_wait)
                extras = waits[: len(waits) - limit]
                if is_pool:
                    for i in range(0, len(extras), 2):
                        es = mybir.InstEventSemaphore(
                            name=f"{inst.name}-sw{i}",
                            sync_info=mybir.SyncInfo(
                                on_wait=extras[i : i + 2], on_update=[]
                            ),
                            engine=inst.engine,
                        )
                        _orig_commit(self, es, lazy_reg_writes)
                else:
                    for i, w in enumerate(extras):
                        nop = mybir.InstNoOp(
                            name=f"{inst.name}-sw{i}",
                            sync_info=mybir.SyncInfo(on_wait=[w], on_update=[]),
                            bass_nofuse=True,
                            engine=inst.engine,
                        )
                        _orig_commit(self, nop, lazy_reg_writes)
                inst.sync_info = mybir.SyncInfo(
                    on_wait=waits[len(waits) - limit :], on_update=list(si.on_update)
                )
            return _orig_commit(self, inst, lazy_reg_writes)

        TileContext._commit_instruction = _commit_split
        TileContext._ant_wait_split = True

    # The stock TileContext tail drain packs every outstanding sem wait into
    # a single InstDrain; this walrus caps non-EventSemaphore instructions at
    # one wait. Emit one drain per outstanding semaphore instead.
    def _drain_and_barrier(self, tick_clock, wait_clock):
        nc = self.nc
        gc = tick_clock.global_clock
        n = len(gc)
        for i in range(n):
            t = gc[i]
            if t <= 0:
                continue
            vc = VectorClock([0] * n)
            vc.require_at_least(i, t)
            d = nc.sync.drain()
            wait_clock.add_sem_waits(d.ins, ScopedClock({None: vc}))
        nc.all_engine_barrier()
        assert self.sems is not None
        popped = nc._tile_sem_poison_stack.pop()
        assert popped is self._sem_poison
        nc.clear_and_free_semaphores(list(self.sems.allocated().values()))
        nc.all_engine_barrier()

    TileContext._drain_and_barrier = _drain_and_barrier


def _build_nc():
    import concourse.bass as bass
    import concourse.mybir as mybir
    from concourse.tile import TileContext

    _install_tile_patches()

    f32 = mybir.dt.float32
    bf16 = mybir.dt.bfloat16
    f8 = mybir.dt.float8e4
    Alu = mybir.AluOpType
    Act = mybir.ActivationFunctionType
    X = mybir.AxisListType

    nc = bass.Bass()
    # alpha planes: row g*128+p, free [o(7), t(8), nl(16), k(16)] fp8
    alpha_d = nc.declare_dram_parameter(
        "alpha_p", [NG * P, NPL * GF], f8, isOutput=False
    )
    # beta pre-transposed on the host into chunk layout:
    # beta_t[el, t*2048 + c*128 + p] = beta[t*128 + p, c*126 + el]
    beta_d = nc.declare_dram_parameter("beta_t", [ECH, T * NCH * P], bf16, isOutput=False)
    theta_d = nc.declare_dram_parameter("theta_t", [P, T], f32, isOutput=False)
    mask_d = nc.declare_dram_parameter("mask_c", [P, NMASK * SW], bf16, isOutput=False)
    mt_d = nc.declare_dram_parameter("mt_c", [ECH, NCH * EDW], bf16, isOutput=False)
    out_d = nc.declare_dram_parameter("loss_part", [1, 1], f32, isOutput=True)

    with TileContext(nc) as tc:
        with (
            tc.tile_pool(name="consts", bufs=1) as cp,
            tc.tile_pool(name="planes", bufs=2) as plp,
            tc.tile_pool(name="tree1", bufs=1) as trp1,
            tc.tile_pool(name="tree2", bufs=2) as trp2,
            tc.tile_pool(name="persist", bufs=1) as pp,
            tc.tile_pool(name="smallp", bufs=2) as sp,
            tc.tile_pool(name="finp", bufs=1) as fp_,
            tc.tile_pool(name="betap", bufs=1) as bp,
            tc.tile_pool(name="ebtp", bufs=2) as ep,
            tc.tile_pool(name="psc", bufs=2, space="PSUM") as psc,
        ):
            # first plane-group DMA gates the DVE pipeline; split it so the
            # tree's first ops can start on the front half
            pl_tiles = [
                plp.tile([P, NPL * GF], bf16, tag="pl", name=f"pl{i}")
                for i in range(2)
            ]
            nc.gpsimd.dma_start(
                pl_tiles[0][:, 0 : 4 * GF], alpha_d[0:P, 0 : 4 * GF]
            )
            nc.gpsimd.dma_start(
                pl_tiles[0][:, 4 * GF : NPL * GF], alpha_d[0:P, 4 * GF : NPL * GF]
            )

            mask_sb = cp.tile([P, NMASK * SW], bf16)
            nc.scalar.dma_start(mask_sb[:, :], mask_d[:, :])
            mt_sb = cp.tile([ECH, NCH * EDW], bf16)
            nc.scalar.dma_start(mt_sb[:, :], mt_d[:, :])
            theta_sb = cp.tile([P, T], f32)
            nc.scalar.dma_start(theta_sb[:, :], theta_d[:, :])
            ones_sb = cp.tile([P, 1], f32)
            nc.vector.memset(ones_sb[:, :], 1.0)

            # prefetch group 1 + the first beta tiles
            nc.gpsimd.dma_start(pl_tiles[1][:, :], alpha_d[P : 2 * P, :])
            b_tiles = [
                bp.tile([ECH, NCH * P], bf16, tag=f"b{t}", name=f"bt{t}")
                for t in range(T)
            ]
            nc.sync.dma_start(b_tiles[0][:, :], beta_d[:, 0 : NCH * P])
            nc.sync.dma_start(b_tiles[1][:, :], beta_d[:, NCH * P : 2 * NCH * P])

            w_sb = pp.tile([P, NG * GF], bf16)    # softmax weights, grouped
            ed_sb = pp.tile([P, T * EDW], f32)    # DP state, zero-init
            tmp_sb = pp.tile([P, T * SW], f32)    # DP step scratch
            nc.vector.memset(ed_sb[:, :], 0.0)

            ed3 = ed_sb.rearrange("p (t k) -> p t k", t=T)
            tmp3 = tmp_sb.rearrange("p (t k) -> p t k", k=SW)

            c_ps = psc.tile([P, T * EDW], f32, tag="c", bufs=1)

            for g in range(NG):
                pl = pl_tiles[g % 2]
                pv = pl.rearrange("p (o f) -> p o f", o=NPL)

                # 4-op max tree, all operands contiguous bf16 (2x mode):
                # l3[i] = max(p2i, p2i+1) for i=0..2, then pairwise
                la = trp1.tile([P, 3 * GF], bf16, tag="la")
                l3 = la.rearrange("p (i f) -> p i f", i=3)
                nc.vector.tensor_tensor(
                    l3[:, :, :], pv[:, 0:6:2, :], pv[:, 1:7:2, :], Alu.max
                )
                lb = trp1.tile([P, GF], bf16, tag="lb")
                nc.vector.tensor_tensor(lb[:, :], l3[:, 0, :], l3[:, 1, :], Alu.max)
                lc = trp1.tile([P, GF], bf16, tag="lc")
                nc.vector.tensor_tensor(lc[:, :], l3[:, 2, :], pv[:, 6, :], Alu.max)
                mxg = trp2.tile([P, GF], bf16, tag="mx")
                nc.vector.tensor_tensor(mxg[:, :], lb[:, :], lc[:, :], Alu.max)

                # kick the next plane DMA as soon as this group's tree has
                # consumed the buffer (program order places it here)
                if g + 2 < NG:
                    ptile = pl_tiles[g % 2]
                    nc.gpsimd.dma_start(
                        ptile[:, :], alpha_d[(g + 2) * P : (g + 3) * P, :]
                    )

                # softmax numerator without max-subtraction (|logits| <~ 6)
                e_sl = w_sb[:, g * GF : (g + 1) * GF]
                nc.scalar.activation(e_sl, mxg[:, :], Act.Exp)
                if g == 0:
                    # zero the invalid window slots of nodes 0..13
                    e30 = e_sl.rearrange("p (t m) -> p t m", m=GW)[
                        :, :, 0 : NMASK * SW
                    ]
                    mask_b = mask_sb.rearrange(
                        "p (o m) -> p o m", o=1
                    ).broadcast_to((P, T, NMASK * SW))
                    nc.vector.tensor_mul(e30, e30, mask_b)

                s_g = sp.tile([P, T * GN], f32, tag="s")
                nc.vector.reduce_sum(
                    s_g[:, :], e_sl.rearrange("p (n k) -> p n k", k=SW), axis=X.X
                )
                lns = sp.tile([P, T * GN], f32, tag="lns")
                nc.scalar.activation(lns[:, :], s_g[:, :], Act.Ln)
                # fused exp(-ln s) + broadcast-expand to [., n, 16] on ACT so
                # the normalize multiply below runs dense bf16 at 2x
                rse = sp.tile([P, GF], bf16, tag="rse")
                nc.scalar.activation(
                    rse.rearrange("p (n k) -> p n k", k=SW),
                    lns.rearrange("p (n o) -> p n o", o=1).broadcast_to(
                        (P, T * GN, SW)
                    ),
                    Act.Exp,
                    scale=-1.0,
                )
                nc.vector.tensor_mul(e_sl, e_sl, rse[:, :])

                # beta stream: two tiles' worth per group (sync ring)
                if 2 * g + 2 < T:
                    nc.sync.dma_start(
                        b_tiles[2 * g + 2][:, :],
                        beta_d[:, (2 * g + 2) * NCH * P : (2 * g + 3) * NCH * P],
                    )
                if 2 * g + 3 < T:
                    nc.sync.dma_start(
                        b_tiles[2 * g + 3][:, :],
                        beta_d[:, (2 * g + 3) * NCH * P : (2 * g + 4) * NCH * P],
                    )

                # DP steps for this group's nodes (all 8 stage slots at once)
                wg = w_sb[:, g * GF : (g + 1) * GF].rearrange(
                    "p (t n k) -> p t n k", t=T, k=SW
                )
                for nl in range(GN):
                    j = g * GN + nl + 2
                    wid = min(j, SW)
                    base = j - wid
                    nc.vector.scalar_tensor_tensor(
                        tmp3[:, :, 0:wid],
                        ed3[:, :, base : base + wid],
                        1.0,
                        wg[:, :, nl, 0:wid],
                        Alu.add,
                        Alu.mult,
                    )
                    nc.vector.reduce_sum(
                        ed3[:, :, j : j + 1], tmp3[:, :, 0:wid], axis=X.X
                    )

            # ---- beta phase (low priority: fills scalar/PE gaps) ----
            for t in range(T):
                eb_t = ep.tile([ECH, NCH * P], bf16, tag="eb")
                nc.scalar.activation(eb_t[:, :], b_tiles[t][:, :], Act.Exp)
                for c in range(NCH):
                    nc.tensor.matmul(
                        c_ps[:, t * EDW : (t + 1) * EDW],
                        eb_t[:, c * P : (c + 1) * P],
                        mt_sb[:, c * EDW : (c + 1) * EDW],
                        start=(c == 0),
                        stop=(c == NCH - 1),
                    )

            # ---- final dots: batched over all 8 stage slots ----
            prod = fp_.tile([P, T * (EDW - 1)], f32, tag="prod")
            q = fp_.tile([P, T], f32, tag="q")
            cv = c_ps.rearrange("p (t k) -> p t k", k=EDW)
            nc.vector.scalar_tensor_tensor(
                prod.rearrange("p (t k) -> p t k", k=EDW - 1),
                ed3[:, :, 0 : EDW - 1],
                0.0,
                cv[:, :, 0 : EDW - 1],
                Alu.add,
                Alu.mult,
            )
            nc.vector.reduce_sum(
                q.rearrange("p (t k) -> p t k", k=1),
                prod.rearrange("p (t k) -> p t k", k=EDW - 1),
                axis=X.X,
            )
            rsb = fp_.tile([P, T], f32, tag="rsb")
            nc.vector.reciprocal(rsb[:, :], cv[:, :, EDW - 1])
            rst = fp_.tile([P, T], f32, tag="rst")
            nc.vector.tensor_mul(rst[:, :], rsb[:, :], theta_sb[:, :])
            acc = fp_.tile([P, T], f32, tag="acc")
            nc.vector.tensor_mul(acc[:, :], q[:, :], rst[:, :])

            # ---- final reduction: 8 cols then 128 partitions ----
            accsum = fp_.tile([P, 1], f32, tag="accsum")
            nc.vector.reduce_sum(accsum[:, :], acc[:, :], axis=X.X)
            out_ps = psc.tile([1, 1], f32, tag="outp", bufs=1)
            nc.tensor.matmul(
                out_ps[:, :], accsum[:, :], ones_sb[:, :], start=True, stop=True
            )
            out_sb = fp_.tile([1, 1], f32, tag="outs")
            nc.scalar.copy(out_sb[:, :], out_ps[:, :])
            nc.sync.dma_start(out_d[:, :], out_sb[:, :])

    return nc


def _get_compiled():
    if "nc" not in _CACHE:
        _CACHE["nc"] = _build_nc()
        _CACHE["consts"] = _host_consts()
    return _CACHE["nc"], _CACHE["consts"]


def _in_maps(alpha, beta, theta):
    import ml_dtypes

    mt, mask = _get_compiled()[1]
    alpha = np.ascontiguousarray(alpha, dtype=np.float32)
    beta = np.ascontiguousarray(beta, dtype=np.float32)
    theta = np.ascontiguousarray(theta, dtype=np.float32)
    alpha_f8 = alpha.astype(ml_dtypes.float8_e4m3)
    beta_bf = beta.astype(ml_dtypes.bfloat16)
    maps = []
    for c in range(N_CORES):
        sl = slice(c * S_CORE, (c + 1) * S_CORE)
        # [t, p, g, nl, k, o] -> [g, p, o, t, nl, k], drop op 7
        A = alpha_f8[sl].reshape(T, P, NG, GN, SW, 8)
        planes = np.ascontiguousarray(A.transpose(2, 1, 5, 0, 3, 4)[:, :, :NPL])
        # [el, t*2048 + ch*128 + p] = beta[t*128 + p, ch*126 + el]
        beta_t = np.ascontiguousarray(
            beta_bf[sl].reshape(T, P, NCH, ECH).transpose(3, 0, 2, 1).reshape(ECH, -1)
        )
        maps.append(
            {
                "alpha_p": planes.reshape(NG * P, NPL * GF),
                "beta_t": beta_t,
                "theta_t": np.ascontiguousarray(theta[sl].reshape(T, P).T),
                "mask_c": mask,
                "mt_c": mt,
            }
        )
    return maps


def _run(alpha, beta, theta, **spmd_kwargs):
    from concourse.bass_utils import run_bass_kernel_spmd

    nc, _ = _get_compiled()
    res = run_bass_kernel_spmd(
        nc, _in_maps(alpha, beta, theta), core_ids=list(range(N_CORES)), **spmd_kwargs
    )
    total = np.float32(0.0)
    for r in res.results:
        total += np.float32(r["loss_part"][0, 0])
    return np.float32(total), res


def kernel(alpha, beta, theta):
    out, _ = _run(alpha, beta, theta)
    return out


# revision 8
# speedup vs baseline: 1.2065x; 1.2065x over previous
"""Trainium2 Bass kernel for the Expected-Depth DP loss.

Computation (see reference):
  - edge_max = max over first 7 of 8 op-logits          [S, 64, 16]
  - w        = masked softmax over the 16-wide window   [S, 64, 16]
  - DP scan:  ed[j] = sum_k w[j,k] * (ed[base+k] + 1),  j = 2..65
  - loss     = sum_s theta[s] * softmax(beta[s]) . (ed[ii] + ed[jj])

Sharding: S=8192 stages split across 8 cores (pure data parallel,
1024 stages/core as 128 partitions x 8 free slots). Per-core partial
losses are summed on the host.

v2 layout/engine choices:
  - alpha staged in HBM as 7 op-major bf16 planes (op 7 unused),
    node-grouped, streamed on the sync HWDGE ring (SWDGE cast-DMA from
    fp8 measured ~2x slower and its descriptor rings degrade DVE 2x).
  - max-of-7 as a 4-instruction bf16 tensor_tensor max tree (2x mode)
    instead of a 1x tensor_reduce.
  - per-node-group pipeline: tree/exp/softmax/DP for nodes [16g,16g+16)
    overlap the next group's plane DMA.
  - the softmax reciprocal is broadcast-expanded on the scalar engine so
    the normalize multiply runs dense bf16 at DVE 2x.
  - beta rides the scalar-engine HWDGE ring in bf16; its exps/matmuls
    are emitted after the group loop so they fill scalar-engine gaps.
"""

import numpy as np

SW = 16          # DP window
NN = 64          # nodes per stage
S = 8192         # stages
E = 2016         # beta edges
P = 128          # SBUF partitions
N_CORES = 8
S_CORE = S // N_CORES        # 1024
T = S_CORE // P              # 8 stage slots per partition
NG = 4                       # node groups
GN = NN // NG                # 16 nodes per group
GW = GN * SW                 # 256 edge_max floats per stage per group
GF = T * GW                  # 2048 free elems per group tile
NPL = 7                      # op planes
EDW = 67                     # ed row stride (66 node slots + 1 pad)
NCH = 16                     # beta column chunks
ECH = E // NCH               # 126 edges per chunk
NMASK = 14                   # nodes with partially-valid windows

_CACHE = {}


def _host_consts():
    import ml_dtypes

    ii, jj = [], []
    for i in range(2, NN + 1):
        for j in range(i + 1, NN + 2):
            ii.append(i)
            jj.append(j)
    ii = np.asarray(ii)
    jj = np.asarray(jj)
    # incidence matrix chunks: mt[e_local, c*67 + k] = [ii==k] + [jj==k],
    # column 66 of each chunk is all ones (softmax denominator)
    mt = np.zeros((NCH, ECH, EDW), np.float32)
    for e in range(E):
        c, el = divmod(e, ECH)
        mt[c, el, ii[e]] += 1.0
        mt[c, el, jj[e]] += 1.0
        mt[c, el, EDW - 1] = 1.0
    mt = np.ascontiguousarray(
        mt.transpose(1, 0, 2).reshape(ECH, NCH * EDW)
    ).astype(ml_dtypes.bfloat16)
    # validity mask for the first 14 nodes (node n: rows k < n+2 valid)
    mask = np.zeros((NMASK, SW), np.float32)
    for n in range(NMASK):
        mask[n, : n + 2] = 1.0
    mask = np.ascontiguousarray(
        np.broadcast_to(mask.reshape(1, NMASK * SW), (P, NMASK * SW))
    ).astype(ml_dtypes.bfloat16)
    return mt, mask


def _install_tile_patches():
    import concourse.mybir as mybir
    from concourse.tile import TileContext
    from concourse.vector_clock import ScopedClock, VectorClock

    # This walrus build rejects TPB instructions carrying more than one sem
    # wait (two for EventSemaphore, zero for Pool-engine non-ES ops), but
    # Tile's wait assignment happily packs 2-3. Split the extras onto
    # single-wait NoOps (ES chunks for Pool) on the same engine.
    if not getattr(TileContext, "_ant_wait_split", False):
        _orig_commit = TileContext._commit_instruction

        def _commit_split(self, inst, lazy_reg_writes=True):
            si = inst.sync_info
            is_es = isinstance(inst, mybir.InstEventSemaphore)
            is_pool = inst.engine == mybir.EngineType.Pool
            limit = 2 if is_es else (0 if is_pool else 1)
            if si is not None and si.on_wait and len(si.on_wait) > limit:
                waits = list(si.on_wait)
                extras = waits[: len(waits) - limit]
                if is_pool:
                    for i in range(0, len(extras), 2):
                        es = mybir.InstEventSemaphore(
                            name=f"{inst.name}-sw{i}",
                            sync_info=mybir.SyncInfo(
                                on_wait=extras[i : i + 2], on_update=[]
                            ),
                            engine=inst.engine,
                        )
                        _orig_commit(self, es, lazy_reg_writes)
                else:
                    for i, w in enumerate(extras):
                        nop = mybir.InstNoOp(
                            name=f"{inst.name}-sw{i}",
                            sync_info=mybir.SyncInfo(on_wait=[w], on_update=[]),
                            bass_nofuse=True,
                            engine=inst.engine,
                        )
                        _orig_commit(self, nop, lazy_reg_writes)
                inst.sync_info = mybir.SyncInfo(
                    on_wait=waits[len(waits) - limit :], on_update=list(si.on_update)
                )
            return _orig_commit(self, inst, lazy_reg_writes)

        TileContext._commit_instruction = _commit_split
        TileContext._ant_wait_split = True

    # The stock TileContext tail drain packs every outstanding sem wait into
    # a single InstDrain; this walrus caps non-EventSemaphore instructions at
    # one wait. Emit one drain per outstanding semaphore instead.
    def _drain_and_barrier(self, tick_clock, wait_clock):
        nc = self.nc
        gc = tick_clock.global_clock
        n = len(gc)
        for i in range(n):
            t = gc[i]
            if t <= 0:
                continue
            vc = VectorClock([0] * n)
            vc.require_at_least(i, t)
            d = nc.sync.drain()
            wait_clock.add_sem_waits(d.ins, ScopedClock({None: vc}))
        nc.all_engine_barrier()
        assert self.sems is not None
        popped = nc._tile_sem_poison_stack.pop()
        assert popped is self._sem_poison
        nc.clear_and_free_semaphores(list(self.sems.allocated().values()))
        nc.all_engine_barrier()

    TileContext._drain_and_barrier = _drain_and_barrier


def _build_nc():
    import concourse.bass as bass
    import concourse.mybir as mybir
    from concourse.tile import TileContext

    _install_tile_patches()

    f32 = mybir.dt.float32
    bf16 = mybir.dt.bfloat16
    f8 = mybir.dt.float8e4
    Alu = mybir.AluOpType
    Act = mybir.ActivationFunctionType
    X = mybir.AxisListType

    nc = bass.Bass()
    # alpha planes: row g*128+p, free [o(7), t(8), nl(16), k(16)] bf16
    alpha_d = nc.declare_dram_parameter(
        "alpha_p", [NG * P, NPL * GF], bf16, isOutput=False
    )
    # beta pre-transposed on the host into chunk layout:
    # beta_t[el, t*2048 + c*128 + p] = beta[t*128 + p, c*126 + el]
    beta_d = nc.declare_dram_parameter("beta_t", [ECH, T * NCH * P], bf16, isOutput=False)
    theta_d = nc.declare_dram_parameter("theta_t", [P, T], f32, isOutput=False)
    mask_d = nc.declare_dram_parameter("mask_c", [P, NMASK * SW], bf16, isOutput=False)
    mt_d = nc.declare_dram_parameter("mt_c", [ECH, NCH * EDW], bf16, isOutput=False)
    out_d = nc.declare_dram_parameter("loss_part", [1, 1], f32, isOutput=True)

    with TileContext(nc) as tc:
        with (
            tc.tile_pool(name="consts", bufs=1) as cp,
            tc.tile_pool(name="planes", bufs=2) as plp,
            tc.tile_pool(name="tree1", bufs=1) as trp1,
            tc.tile_pool(name="tree2", bufs=2) as trp2,
            tc.tile_pool(name="persist", bufs=1) as pp,
            tc.tile_pool(name="smallp", bufs=2) as sp,
            tc.tile_pool(name="finp", bufs=1) as fp_,
            tc.tile_pool(name="betap", bufs=1) as bp,
            tc.tile_pool(name="ebtp", bufs=2) as ep,
            tc.tile_pool(name="psc", bufs=2, space="PSUM") as psc,
        ):
            # first plane-group DMA gates the DVE pipeline; split it so the
            # tree's first ops can start on the front half
            pl_tiles = [
                plp.tile([P, NPL * GF], bf16, tag="pl", name=f"pl{i}")
                for i in range(2)
            ]
            nc.sync.dma_start(
                pl_tiles[0][:, 0 : 4 * GF], alpha_d[0:P, 0 : 4 * GF]
            )
            nc.sync.dma_start(
                pl_tiles[0][:, 4 * GF : NPL * GF], alpha_d[0:P, 4 * GF : NPL * GF]
            )

            mask_sb = cp.tile([P, NMASK * SW], bf16)
            nc.scalar.dma_start(mask_sb[:, :], mask_d[:, :])
            mt_sb = cp.tile([ECH, NCH * EDW], bf16)
            nc.scalar.dma_start(mt_sb[:, :], mt_d[:, :])
            theta_sb = cp.tile([P, T], f32)
            nc.scalar.dma_start(theta_sb[:, :], theta_d[:, :])
            ones_sb = cp.tile([P, 1], f32)
            nc.vector.memset(ones_sb[:, :], 1.0)

            # prefetch group 1 + the first beta tiles
            nc.sync.dma_start(pl_tiles[1][:, :], alpha_d[P : 2 * P, :])
            b_tiles = [
                bp.tile([ECH, NCH * P], bf16, tag=f"b{t}", name=f"bt{t}")
                for t in range(T)
            ]
            nc.scalar.dma_start(b_tiles[0][:, :], beta_d[:, 0 : NCH * P])
            nc.scalar.dma_start(b_tiles[1][:, :], beta_d[:, NCH * P : 2 * NCH * P])

            w_sb = pp.tile([P, NG * GF], bf16)    # softmax weights, grouped
            ed_sb = pp.tile([P, T * EDW], f32)    # DP state, zero-init
            tmp_sb = pp.tile([P, T * SW], f32)    # DP step scratch
            nc.vector.memset(ed_sb[:, :], 0.0)

            ed3 = ed_sb.rearrange("p (t k) -> p t k", t=T)
            tmp3 = tmp_sb.rearrange("p (t k) -> p t k", k=SW)

            c_ps = psc.tile([P, T * EDW], f32, tag="c", bufs=1)

            for g in range(NG):
                pl = pl_tiles[g % 2]
                pv = pl.rearrange("p (o f) -> p o f", o=NPL)

                # 4-op max tree, all operands contiguous bf16 (2x mode):
                # l3[i] = max(p2i, p2i+1) for i=0..2, then pairwise
                la = trp1.tile([P, 3 * GF], bf16, tag="la")
                l3 = la.rearrange("p (i f) -> p i f", i=3)
                nc.vector.tensor_tensor(
                    l3[:, :, :], pv[:, 0:6:2, :], pv[:, 1:7:2, :], Alu.max
                )
                lb = trp1.tile([P, GF], bf16, tag="lb")
                nc.vector.tensor_tensor(lb[:, :], l3[:, 0, :], l3[:, 1, :], Alu.max)
                lc = trp1.tile([P, GF], bf16, tag="lc")
                nc.vector.tensor_tensor(lc[:, :], l3[:, 2, :], pv[:, 6, :], Alu.max)
                mxg = trp2.tile([P, GF], bf16, tag="mx")
                nc.vector.tensor_tensor(mxg[:, :], lb[:, :], lc[:, :], Alu.max)

                # kick the next plane DMA as soon as this group's tree has
                # consumed the buffer (program order places it here)
                if g + 2 < NG:
                    ptile = pl_tiles[g % 2]
                    nc.sync.dma_start(
                        ptile[:, :], alpha_d[(g + 2) * P : (g + 3) * P, :]
                    )

                # softmax numerator without max-subtraction (|logits| <~ 6)
                e_sl = w_sb[:, g * GF : (g + 1) * GF]
                nc.scalar.activation(e_sl, mxg[:, :], Act.Exp)
                if g == 0:
                    # zero the invalid window slots of nodes 0..13
                    e30 = e_sl.rearrange("p (t m) -> p t m", m=GW)[
                        :, :, 0 : NMASK * SW
                    ]
                    mask_b = mask_sb.rearrange(
                        "p (o m) -> p o m", o=1
                    ).broadcast_to((P, T, NMASK * SW))
                    nc.vector.tensor_mul(e30, e30, mask_b)

                s_g = sp.tile([P, T * GN], f32, tag="s")
                nc.vector.reduce_sum(
                    s_g[:, :], e_sl.rearrange("p (n k) -> p n k", k=SW), axis=X.X
                )
                lns = sp.tile([P, T * GN], f32, tag="lns")
                nc.scalar.activation(lns[:, :], s_g[:, :], Act.Ln)
                # fused exp(-ln s) + broadcast-expand to [., n, 16] on ACT so
                # the normalize multiply below runs dense bf16 at 2x
                rse = sp.tile([P, GF], bf16, tag="rse")
                nc.scalar.activation(
                    rse.rearrange("p (n k) -> p n k", k=SW),
                    lns.rearrange("p (n o) -> p n o", o=1).broadcast_to(
                        (P, T * GN, SW)
                    ),
                    Act.Exp,
                    scale=-1.0,
                )
                nc.vector.tensor_mul(e_sl, e_sl, rse[:, :])

                # beta stream: two tiles' worth per group (sync ring)
                if 2 * g + 2 < T:
                    nc.scalar.dma_start(
                        b_tiles[2 * g + 2][:, :],
                        beta_d[:, (2 * g + 2) * NCH * P : (2 * g + 3) * NCH * P],
                    )
                if 2 * g + 3 < T:
                    nc.scalar.dma_start(
                        b_tiles[2 * g + 3][:, :],
                        beta_d[:, (2 * g + 3) * NCH * P : (2 * g + 4) * NCH * P],
                    )

                # DP steps for this group's nodes (all 8 stage slots at once)
                wg = w_sb[:, g * GF : (g + 1) * GF].rearrange(
                    "p (t n k) -> p t n k", t=T, k=SW
                )
                for nl in range(GN):
                    j = g * GN + nl + 2
                    wid = min(j, SW)
                    base = j - wid
                    nc.vector.scalar_tensor_tensor(
                        tmp3[:, :, 0:wid],
                        ed3[:, :, base : base + wid],
                        1.0,
                        wg[:, :, nl, 0:wid],
                        Alu.add,
                        Alu.mult,
                    )
                    nc.vector.reduce_sum(
                        ed3[:, :, j : j + 1], tmp3[:, :, 0:wid], axis=X.X
                    )

            # ---- beta phase (low priority: fills scalar/PE gaps) ----
            for t in range(T):
                eb_t = ep.tile([ECH, NCH * P], bf16, tag="eb")
                nc.scalar.activation(eb_t[:, :], b_tiles[t][:, :], Act.Exp)
                for c in range(NCH):
                    nc.tensor.matmul(
                        c_ps[:, t * EDW : (t + 1) * EDW],
                        eb_t[:, c * P : (c + 1) * P],
                        mt_sb[:, c * EDW : (c + 1) * EDW],
                        start=(c == 0),
                        stop=(c == NCH - 1),
                    )

            # ---- final dots: batched over all 8 stage slots ----
            prod = fp_.tile([P, T * (EDW - 1)], f32, tag="prod")
            q = fp_.tile([P, T], f32, tag="q")
            cv = c_ps.rearrange("p (t k) -> p t k", k=EDW)
            nc.vector.scalar_tensor_tensor(
                prod.rearrange("p (t k) -> p t k", k=EDW - 1),
                ed3[:, :, 0 : EDW - 1],
                0.0,
                cv[:, :, 0 : EDW - 1],
                Alu.add,
                Alu.mult,
            )
            nc.vector.reduce_sum(
                q.rearrange("p (t k) -> p t k", k=1),
                prod.rearrange("p (t k) -> p t k", k=EDW - 1),
                axis=X.X,
            )
            rsb = fp_.tile([P, T], f32, tag="rsb")
            nc.vector.reciprocal(rsb[:, :], cv[:, :, EDW - 1])
            rst = fp_.tile([P, T], f32, tag="rst")
            nc.vector.tensor_mul(rst[:, :], rsb[:, :], theta_sb[:, :])
            acc = fp_.tile([P, T], f32, tag="acc")
            nc.vector.tensor_mul(acc[:, :], q[:, :], rst[:, :])

            # ---- final reduction: 8 cols then 128 partitions ----
            accsum = fp_.tile([P, 1], f32, tag="accsum")
            nc.vector.reduce_sum(accsum[:, :], acc[:, :], axis=X.X)
            out_ps = psc.tile([1, 1], f32, tag="outp", bufs=1)
            nc.tensor.matmul(
                out_ps[:, :], accsum[:, :], ones_sb[:, :], start=True, stop=True
            )
            out_sb = fp_.tile([1, 1], f32, tag="outs")
            nc.scalar.copy(out_sb[:, :], out_ps[:, :])
            nc.sync.dma_start(out_d[:, :], out_sb[:, :])

    return nc


def _get_compiled():
    if "nc" not in _CACHE:
        _CACHE["nc"] = _build_nc()
        _CACHE["consts"] = _host_consts()
    return _CACHE["nc"], _CACHE["consts"]


def _in_maps(alpha, beta, theta):
    import ml_dtypes

    mt, mask = _get_compiled()[1]
    alpha = np.ascontiguousarray(alpha, dtype=np.float32)
    beta = np.ascontiguousarray(beta, dtype=np.float32)
    theta = np.ascontiguousarray(theta, dtype=np.float32)
    alpha_bf = alpha.astype(ml_dtypes.bfloat16)
    beta_bf = beta.astype(ml_dtypes.bfloat16)
    maps = []
    for c in range(N_CORES):
        sl = slice(c * S_CORE, (c + 1) * S_CORE)
        # [t, p, g, nl, k, o] -> [g, p, o, t, nl, k], drop op 7
        A = alpha_bf[sl].reshape(T, P, NG, GN, SW, 8)
        planes = np.ascontiguousarray(A.transpose(2, 1, 5, 0, 3, 4)[:, :, :NPL])
        # [el, t*2048 + ch*128 + p] = beta[t*128 + p, ch*126 + el]
        beta_t = np.ascontiguousarray(
            beta_bf[sl].reshape(T, P, NCH, ECH).transpose(3, 0, 2, 1).reshape(ECH, -1)
        )
        maps.append(
            {
                "alpha_p": planes.reshape(NG * P, NPL * GF),
                "beta_t": beta_t,
                "theta_t": np.ascontiguousarray(theta[sl].reshape(T, P).T),
                "mask_c": mask,
                "mt_c": mt,
            }
        )
    return maps


def _run(alpha, beta, theta, **spmd_kwargs):
    from concourse.bass_utils import run_bass_kernel_spmd

    nc, _ = _get_compiled()
    res = run_bass_kernel_spmd(
        nc, _in_maps(alpha, beta, theta), core_ids=list(range(N_CORES)), **spmd_kwargs
    )
    total = np.float32(0.0)
    for r in res.results:
        total += np.float32(r["loss_part"][0, 0])
    return np.float32(total), res


def kernel(alpha, beta, theta):
    out, _ = _run(alpha, beta, theta)
    return out


# revision 9
# speedup vs baseline: 1.2958x; 1.0741x over previous
"""Trainium2 Bass kernel for the Expected-Depth DP loss.

Computation (see reference):
  - edge_max = max over first 7 of 8 op-logits          [S, 64, 16]
  - w        = masked softmax over the 16-wide window   [S, 64, 16]
  - DP scan:  ed[j] = sum_k w[j,k] * (ed[base+k] + 1),  j = 2..65
  - loss     = sum_s theta[s] * softmax(beta[s]) . (ed[ii] + ed[jj])

Sharding: S=8192 stages split across 8 cores (pure data parallel,
1024 stages/core as 128 partitions x 8 free slots). Per-core partial
losses are summed on the host.

v2 layout/engine choices:
  - alpha staged in HBM as 7 op-major bf16 planes (op 7 unused),
    node-grouped, streamed on the sync HWDGE ring (SWDGE cast-DMA from
    fp8 measured ~2x slower and its descriptor rings degrade DVE 2x).
  - max-of-7 as a 4-instruction bf16 tensor_tensor max tree (2x mode)
    instead of a 1x tensor_reduce.
  - per-node-group pipeline: tree/exp/softmax/DP for nodes [16g,16g+16)
    overlap the next group's plane DMA.
  - the softmax reciprocal is broadcast-expanded on the scalar engine so
    the normalize multiply runs dense bf16 at DVE 2x.
  - beta rides the scalar-engine HWDGE ring in bf16; its exps/matmuls
    are emitted after the group loop so they fill scalar-engine gaps.
"""

import numpy as np

SW = 16          # DP window
NN = 64          # nodes per stage
S = 8192         # stages
E = 2016         # beta edges
P = 128          # SBUF partitions
N_CORES = 8
S_CORE = S // N_CORES        # 1024
T = S_CORE // P              # 8 stage slots per partition
NG = 4                       # node groups
GN = NN // NG                # 16 nodes per group
GW = GN * SW                 # 256 edge_max floats per stage per group
GF = T * GW                  # 2048 free elems per group tile
NPL = 7                      # op planes
EDW = 67                     # ed row stride (66 node slots + 1 pad)
NCH = 16                     # beta column chunks
ECH = E // NCH               # 126 edges per chunk
NMASK = 14                   # nodes with partially-valid windows

_CACHE = {}


def _host_consts():
    import ml_dtypes

    ii, jj = [], []
    for i in range(2, NN + 1):
        for j in range(i + 1, NN + 2):
            ii.append(i)
            jj.append(j)
    ii = np.asarray(ii)
    jj = np.asarray(jj)
    # incidence matrix chunks: mt[e_local, c*67 + k] = [ii==k] + [jj==k],
    # column 66 of each chunk is all ones (softmax denominator)
    mt = np.zeros((NCH, ECH, EDW), np.float32)
    for e in range(E):
        c, el = divmod(e, ECH)
        mt[c, el, ii[e]] += 1.0
        mt[c, el, jj[e]] += 1.0
        mt[c, el, EDW - 1] = 1.0
    mt = np.ascontiguousarray(
        mt.transpose(1, 0, 2).reshape(ECH, NCH * EDW)
    ).astype(ml_dtypes.bfloat16)
    # validity mask for the first 14 nodes (node n: rows k < n+2 valid)
    mask = np.zeros((NMASK, SW), np.float32)
    for n in range(NMASK):
        mask[n, : n + 2] = 1.0
    mask = np.ascontiguousarray(
        np.broadcast_to(mask.reshape(1, NMASK * SW), (P, NMASK * SW))
    ).astype(ml_dtypes.bfloat16)
    return mt, mask


def _install_tile_patches():
    import concourse.mybir as mybir
    from concourse.tile import TileContext
    from concourse.vector_clock import ScopedClock, VectorClock

    # This walrus build rejects TPB instructions carrying more than one sem
    # wait (two for EventSemaphore, zero for Pool-engine non-ES ops), but
    # Tile's wait assignment happily packs 2-3. Split the extras onto
    # single-wait NoOps (ES chunks for Pool) on the same engine.
    if not getattr(TileContext, "_ant_wait_split", False):
        _orig_commit = TileContext._commit_instruction

        def _commit_split(self, inst, lazy_reg_writes=True):
            si = inst.sync_info
            is_es = isinstance(inst, mybir.InstEventSemaphore)
            is_pool = inst.engine == mybir.EngineType.Pool
            limit = 2 if is_es else (0 if is_pool else 1)
            if si is not None and si.on_wait and len(si.on_wait) > limit:
                waits = list(si.on_wait)
                extras = waits[: len(waits) - limit]
                if is_pool:
                    for i in range(0, len(extras), 2):
                        es = mybir.InstEventSemaphore(
                            name=f"{inst.name}-sw{i}",
                            sync_info=mybir.SyncInfo(
                                on_wait=extras[i : i + 2], on_update=[]
                            ),
                            engine=inst.engine,
                        )
                        _orig_commit(self, es, lazy_reg_writes)
                else:
                    for i, w in enumerate(extras):
                        nop = mybir.InstNoOp(
                            name=f"{inst.name}-sw{i}",
                            sync_info=mybir.SyncInfo(on_wait=[w], on_update=[]),
                            bass_nofuse=True,
                            engine=inst.engine,
                        )
                        _orig_commit(self, nop, lazy_reg_writes)
                inst.sync_info = mybir.SyncInfo(
                    on_wait=waits[len(waits) - limit :], on_update=list(si.on_update)
                )
            return _orig_commit(self, inst, lazy_reg_writes)

        TileContext._commit_instruction = _commit_split
        TileContext._ant_wait_split = True

    # The stock TileContext tail drain packs every outstanding sem wait into
    # a single InstDrain; this walrus caps non-EventSemaphore instructions at
    # one wait. Emit one drain per outstanding semaphore instead.
    def _drain_and_barrier(self, tick_clock, wait_clock):
        nc = self.nc
        gc = tick_clock.global_clock
        n = len(gc)
        for i in range(n):
            t = gc[i]
            if t <= 0:
                continue
            vc = VectorClock([0] * n)
            vc.require_at_least(i, t)
            d = nc.sync.drain()
            wait_clock.add_sem_waits(d.ins, ScopedClock({None: vc}))
        nc.all_engine_barrier()
        assert self.sems is not None
        popped = nc._tile_sem_poison_stack.pop()
        assert popped is self._sem_poison
        nc.clear_and_free_semaphores(list(self.sems.allocated().values()))
        nc.all_engine_barrier()

    TileContext._drain_and_barrier = _drain_and_barrier


def _build_nc():
    import concourse.bass as bass
    import concourse.mybir as mybir
    from concourse.tile import TileContext

    _install_tile_patches()

    f32 = mybir.dt.float32
    bf16 = mybir.dt.bfloat16
    f8 = mybir.dt.float8e4
    Alu = mybir.AluOpType
    Act = mybir.ActivationFunctionType
    X = mybir.AxisListType

    nc = bass.Bass()
    # alpha planes: row g*128+p, free [o(7), t(8), nl(16), k(16)] bf16
    alpha_d = nc.declare_dram_parameter(
        "alpha_p", [NG * P, NPL * GF], bf16, isOutput=False
    )
    # beta pre-transposed on the host into chunk layout:
    # beta_t[el, t*2048 + c*128 + p] = beta[t*128 + p, c*126 + el]
    beta_d = nc.declare_dram_parameter("beta_t", [ECH, T * NCH * P], bf16, isOutput=False)
    theta_d = nc.declare_dram_parameter("theta_t", [P, T], f32, isOutput=False)
    mask_d = nc.declare_dram_parameter("mask_c", [P, NMASK * SW], bf16, isOutput=False)
    mt_d = nc.declare_dram_parameter("mt_c", [ECH, NCH * EDW], bf16, isOutput=False)
    out_d = nc.declare_dram_parameter("loss_part", [1, 1], f32, isOutput=True)

    with TileContext(nc) as tc:
        with (
            tc.tile_pool(name="consts", bufs=1) as cp,
            tc.tile_pool(name="planes", bufs=3) as plp,
            tc.tile_pool(name="tree1", bufs=1) as trp1,
            tc.tile_pool(name="tree2", bufs=2) as trp2,
            tc.tile_pool(name="persist", bufs=1) as pp,
            tc.tile_pool(name="smallp", bufs=2) as sp,
            tc.tile_pool(name="finp", bufs=1) as fp_,
            tc.tile_pool(name="betap", bufs=1) as bp,
            tc.tile_pool(name="ebtp", bufs=2) as ep,
            tc.tile_pool(name="psc", bufs=2, space="PSUM") as psc,
        ):
            # first plane-group DMA gates the DVE pipeline; split it so the
            # tree's first ops can start on the front half
            pl_tiles = [
                plp.tile([P, NPL * GF], bf16, tag="pl", name=f"pl{i}")
                for i in range(3)
            ]
            nc.sync.dma_start(
                pl_tiles[0][:, 0 : 4 * GF], alpha_d[0:P, 0 : 4 * GF]
            )
            nc.sync.dma_start(
                pl_tiles[0][:, 4 * GF : NPL * GF], alpha_d[0:P, 4 * GF : NPL * GF]
            )

            mask_sb = cp.tile([P, NMASK * SW], bf16)
            nc.scalar.dma_start(mask_sb[:, :], mask_d[:, :])
            mt_sb = cp.tile([ECH, NCH * EDW], bf16)
            nc.scalar.dma_start(mt_sb[:, :], mt_d[:, :])
            theta_sb = cp.tile([P, T], f32)
            nc.scalar.dma_start(theta_sb[:, :], theta_d[:, :])
            ones_sb = cp.tile([P, 1], f32)
            nc.vector.memset(ones_sb[:, :], 1.0)

            # prefetch groups 1-2 + the first beta tiles
            nc.sync.dma_start(pl_tiles[1][:, :], alpha_d[P : 2 * P, :])
            nc.sync.dma_start(pl_tiles[2][:, :], alpha_d[2 * P : 3 * P, :])
            b_tiles = [
                bp.tile([ECH, NCH * P], bf16, tag=f"b{t}", name=f"bt{t}")
                for t in range(T)
            ]
            nc.scalar.dma_start(b_tiles[0][:, :], beta_d[:, 0 : NCH * P])
            nc.scalar.dma_start(b_tiles[1][:, :], beta_d[:, NCH * P : 2 * NCH * P])

            w_sb = pp.tile([P, NG * GF], bf16)    # softmax weights, grouped
            ed_sb = pp.tile([P, T * EDW], f32)    # DP state, zero-init
            tmp_sb = pp.tile([P, T * SW], f32)    # DP step scratch
            nc.vector.memset(ed_sb[:, :], 0.0)

            ed3 = ed_sb.rearrange("p (t k) -> p t k", t=T)
            tmp3 = tmp_sb.rearrange("p (t k) -> p t k", k=SW)

            c_ps = psc.tile([P, T * EDW], f32, tag="c", bufs=1)

            for g in range(NG):
                pl = pl_tiles[g % 3]
                pv = pl.rearrange("p (o f) -> p o f", o=NPL)

                # max tree over 7 planes, all operands dense bf16 (2x mode)
                mxg = trp2.tile([P, GF], bf16, tag="mx")
                if g == 0:
                    # start on the first DMA half (planes 0-3)
                    la = trp1.tile([P, 2 * GF], bf16, tag="la")
                    l2 = la.rearrange("p (i f) -> p i f", i=2)
                    nc.vector.tensor_tensor(
                        l2[:, :, :], pv[:, 0:2, :], pv[:, 2:4, :], Alu.max
                    )
                    lb = trp1.tile([P, GF], bf16, tag="lb")
                    nc.vector.tensor_tensor(
                        lb[:, :], l2[:, 0, :], l2[:, 1, :], Alu.max
                    )
                    lc = trp1.tile([P, GF], bf16, tag="lc")
                    nc.vector.tensor_tensor(lc[:, :], pv[:, 4, :], pv[:, 5, :], Alu.max)
                    nc.vector.tensor_tensor(lc[:, :], lc[:, :], pv[:, 6, :], Alu.max)
                    nc.vector.tensor_tensor(mxg[:, :], lb[:, :], lc[:, :], Alu.max)
                else:
                    # L1: max(planes 0-2, planes 3-5) in one dense op
                    la = trp1.tile([P, 3 * GF], bf16, tag="la")
                    l3 = la.rearrange("p (i f) -> p i f", i=3)
                    nc.vector.tensor_tensor(
                        l3[:, :, :], pv[:, 0:3, :], pv[:, 3:6, :], Alu.max
                    )
                    lb = trp1.tile([P, GF], bf16, tag="lb")
                    nc.vector.tensor_tensor(
                        lb[:, :], l3[:, 0, :], l3[:, 1, :], Alu.max
                    )
                    lc = trp1.tile([P, GF], bf16, tag="lc")
                    nc.vector.tensor_tensor(lc[:, :], l3[:, 2, :], pv[:, 6, :], Alu.max)
                    nc.vector.tensor_tensor(mxg[:, :], lb[:, :], lc[:, :], Alu.max)

                # reuse buffer 0 for group 3 once group 0's tree is done
                if g == 0:
                    nc.sync.dma_start(
                        pl_tiles[0][:, :], alpha_d[3 * P : 4 * P, :]
                    )

                # softmax numerator without max-subtraction (|logits| <~ 6)
                e_sl = w_sb[:, g * GF : (g + 1) * GF]
                nc.scalar.activation(e_sl, mxg[:, :], Act.Exp)
                if g == 0:
                    # zero the invalid window slots of nodes 0..13
                    e30 = e_sl.rearrange("p (t m) -> p t m", m=GW)[
                        :, :, 0 : NMASK * SW
                    ]
                    mask_b = mask_sb.rearrange(
                        "p (o m) -> p o m", o=1
                    ).broadcast_to((P, T, NMASK * SW))
                    nc.vector.tensor_mul(e30, e30, mask_b)

                s_g = sp.tile([P, T * GN], f32, tag="s")
                nc.vector.reduce_sum(
                    s_g[:, :], e_sl.rearrange("p (n k) -> p n k", k=SW), axis=X.X
                )
                lns = sp.tile([P, T * GN], f32, tag="lns")
                nc.scalar.activation(lns[:, :], s_g[:, :], Act.Ln)
                # fused exp(-ln s) + broadcast-expand to [., n, 16] on ACT so
                # the normalize multiply below runs dense bf16 at 2x
                rse = sp.tile([P, GF], bf16, tag="rse")
                nc.scalar.activation(
                    rse.rearrange("p (n k) -> p n k", k=SW),
                    lns.rearrange("p (n o) -> p n o", o=1).broadcast_to(
                        (P, T * GN, SW)
                    ),
                    Act.Exp,
                    scale=-1.0,
                )
                nc.vector.tensor_mul(e_sl, e_sl, rse[:, :])

                # beta stream: two tiles' worth per group (sync ring)
                if 2 * g + 2 < T:
                    nc.scalar.dma_start(
                        b_tiles[2 * g + 2][:, :],
                        beta_d[:, (2 * g + 2) * NCH * P : (2 * g + 3) * NCH * P],
                    )
                if 2 * g + 3 < T:
                    nc.scalar.dma_start(
                        b_tiles[2 * g + 3][:, :],
                        beta_d[:, (2 * g + 3) * NCH * P : (2 * g + 4) * NCH * P],
                    )

                # DP steps for this group's nodes (all 8 stage slots at once)
                wg = w_sb[:, g * GF : (g + 1) * GF].rearrange(
                    "p (t n k) -> p t n k", t=T, k=SW
                )
                for nl in range(GN):
                    j = g * GN + nl + 2
                    wid = min(j, SW)
                    base = j - wid
                    nc.vector.scalar_tensor_tensor(
                        tmp3[:, :, 0:wid],
                        ed3[:, :, base : base + wid],
                        1.0,
                        wg[:, :, nl, 0:wid],
                        Alu.add,
                        Alu.mult,
                    )
                    nc.vector.reduce_sum(
                        ed3[:, :, j : j + 1], tmp3[:, :, 0:wid], axis=X.X
                    )

            # ---- beta phase (low priority: fills scalar/PE gaps) ----
            for t in range(T):
                eb_t = ep.tile([ECH, NCH * P], bf16, tag="eb")
                nc.scalar.activation(eb_t[:, :], b_tiles[t][:, :], Act.Exp)
                for c in range(NCH):
                    nc.tensor.matmul(
                        c_ps[:, t * EDW : (t + 1) * EDW],
                        eb_t[:, c * P : (c + 1) * P],
                        mt_sb[:, c * EDW : (c + 1) * EDW],
                        start=(c == 0),
                        stop=(c == NCH - 1),
                    )

            # ---- final dots: batched over all 8 stage slots ----
            prod = fp_.tile([P, T * (EDW - 1)], f32, tag="prod")
            q = fp_.tile([P, T], f32, tag="q")
            cv = c_ps.rearrange("p (t k) -> p t k", k=EDW)
            nc.vector.scalar_tensor_tensor(
                prod.rearrange("p (t k) -> p t k", k=EDW - 1),
                ed3[:, :, 0 : EDW - 1],
                0.0,
                cv[:, :, 0 : EDW - 1],
                Alu.add,
                Alu.mult,
            )
            nc.vector.reduce_sum(
                q.rearrange("p (t k) -> p t k", k=1),
                prod.rearrange("p (t k) -> p t k", k=EDW - 1),
                axis=X.X,
            )
            rsb = fp_.tile([P, T], f32, tag="rsb")
            nc.vector.reciprocal(rsb[:, :], cv[:, :, EDW - 1])
            rst = fp_.tile([P, T], f32, tag="rst")
            nc.vector.tensor_mul(rst[:, :], rsb[:, :], theta_sb[:, :])
            acc = fp_.tile([P, T], f32, tag="acc")
            nc.vector.tensor_mul(acc[:, :], q[:, :], rst[:, :])

            # ---- final reduction: 8 cols then 128 partitions ----
            accsum = fp_.tile([P, 1], f32, tag="accsum")
            nc.vector.reduce_sum(accsum[:, :], acc[:, :], axis=X.X)
            out_ps = psc.tile([1, 1], f32, tag="outp", bufs=1)
            nc.tensor.matmul(
                out_ps[:, :], accsum[:, :], ones_sb[:, :], start=True, stop=True
            )
            out_sb = fp_.tile([1, 1], f32, tag="outs")
            nc.scalar.copy(out_sb[:, :], out_ps[:, :])
            nc.sync.dma_start(out_d[:, :], out_sb[:, :])

    return nc


def _get_compiled():
    if "nc" not in _CACHE:
        _CACHE["nc"] = _build_nc()
        _CACHE["consts"] = _host_consts()
    return _CACHE["nc"], _CACHE["consts"]


def _in_maps(alpha, beta, theta):
    import ml_dtypes

    mt, mask = _get_compiled()[1]
    alpha = np.ascontiguousarray(alpha, dtype=np.float32)
    beta = np.ascontiguousarray(beta, dtype=np.float32)
    theta = np.ascontiguousarray(theta, dtype=np.float32)
    alpha_bf = alpha.astype(ml_dtypes.bfloat16)
    beta_bf = beta.astype(ml_dtypes.bfloat16)
    maps = []
    for c in range(N_CORES):
        sl = slice(c * S_CORE, (c + 1) * S_CORE)
        # [t, p, g, nl, k, o] -> [g, p, o, t, nl, k], drop op 7
        A = alpha_bf[sl].reshape(T, P, NG, GN, SW, 8)
        planes = np.ascontiguousarray(A.transpose(2, 1, 5, 0, 3, 4)[:, :, :NPL])
        # [el, t*2048 + ch*128 + p] = beta[t*128 + p, ch*126 + el]
        beta_t = np.ascontiguousarray(
            beta_bf[sl].reshape(T, P, NCH, ECH).transpose(3, 0, 2, 1).reshape(ECH, -1)
        )
        maps.append(
            {
                "alpha_p": planes.reshape(NG * P, NPL * GF),
                "beta_t": beta_t,
                "theta_t": np.ascontiguousarray(theta[sl].reshape(T, P).T),
                "mask_c": mask,
                "mt_c": mt,
            }
        )
    return maps


def _run(alpha, beta, theta, **spmd_kwargs):
    from concourse.bass_utils import run_bass_kernel_spmd

    nc, _ = _get_compiled()
    res = run_bass_kernel_spmd(
        nc, _in_maps(alpha, beta, theta), core_ids=list(range(N_CORES)), **spmd_kwargs
    )
    total = np.float32(0.0)
    for r in res.results:
        total += np.float32(r["loss_part"][0, 0])
    return np.float32(total), res


def kernel(alpha, beta, theta):
    out, _ = _run(alpha, beta, theta)
    return out


# revision 10
# speedup vs baseline: 1.5198x; 1.1728x over previous
"""Trainium2 Bass kernel for the Expected-Depth DP loss.

Computation (see reference):
  - edge_max = max over first 7 of 8 op-logits          [S, 64, 16]
  - w        = masked softmax over the 16-wide window   [S, 64, 16]
  - DP scan:  ed[j] = sum_k w[j,k] * (ed[base+k] + 1),  j = 2..65
  - loss     = sum_s theta[s] * softmax(beta[s]) . (ed[ii] + ed[jj])

Sharding: S=8192 stages split across 8 cores (pure data parallel,
1024 stages/core as 128 partitions x 8 free slots). Per-core partial
losses are summed on the host.

v2 layout/engine choices:
  - alpha staged in HBM as 7 op-major bf16 planes (op 7 unused),
    node-grouped, streamed on the sync HWDGE ring (SWDGE cast-DMA from
    fp8 measured ~2x slower and its descriptor rings degrade DVE 2x).
  - max-of-7 as a 4-instruction bf16 tensor_tensor max tree (2x mode)
    instead of a 1x tensor_reduce.
  - per-node-group pipeline: tree/exp/softmax/DP for nodes [16g,16g+16)
    overlap the next group's plane DMA.
  - the softmax reciprocal is broadcast-expanded on the scalar engine so
    the normalize multiply runs dense bf16 at DVE 2x.
  - beta rides the scalar-engine HWDGE ring in bf16; its exps/matmuls
    are emitted after the group loop so they fill scalar-engine gaps.
"""

import numpy as np

SW = 16          # DP window
NN = 64          # nodes per stage
S = 8192         # stages
E = 2016         # beta edges
P = 128          # SBUF partitions
N_CORES = 8
S_CORE = S // N_CORES        # 1024
T = S_CORE // P              # 8 stage slots per partition
NG = 4                       # node groups
GN = NN // NG                # 16 nodes per group
GW = GN * SW                 # 256 edge_max floats per stage per group
GF = T * GW                  # 2048 free elems per group tile
NPL = 7                      # op planes
EDW = 67                     # ed row stride (66 node slots + 1 pad)
NCH = 16                     # beta column chunks
ECH = E // NCH               # 126 edges per chunk
NMASK = 14                   # nodes with partially-valid windows

_CACHE = {}


def _host_consts():
    import ml_dtypes

    ii, jj = [], []
    for i in range(2, NN + 1):
        for j in range(i + 1, NN + 2):
            ii.append(i)
            jj.append(j)
    ii = np.asarray(ii)
    jj = np.asarray(jj)
    # incidence matrix chunks: mt[e_local, c*67 + k] = [ii==k] + [jj==k],
    # column 66 of each chunk is all ones (softmax denominator)
    mt = np.zeros((NCH, ECH, EDW), np.float32)
    for e in range(E):
        c, el = divmod(e, ECH)
        mt[c, el, ii[e]] += 1.0
        mt[c, el, jj[e]] += 1.0
        mt[c, el, EDW - 1] = 1.0
    mt = np.ascontiguousarray(
        mt.transpose(1, 0, 2).reshape(ECH, NCH * EDW)
    ).astype(ml_dtypes.bfloat16)
    # validity mask for the first 14 nodes (node n: rows k < n+2 valid)
    mask = np.zeros((NMASK, SW), np.float32)
    for n in range(NMASK):
        mask[n, : n + 2] = 1.0
    mask = np.ascontiguousarray(
        np.broadcast_to(mask.reshape(1, NMASK * SW), (P, NMASK * SW))
    ).astype(ml_dtypes.bfloat16)
    return mt, mask


def _install_tile_patches():
    import concourse.mybir as mybir
    from concourse.tile import TileContext
    from concourse.vector_clock import ScopedClock, VectorClock

    # This walrus build rejects TPB instructions carrying more than one sem
    # wait (two for EventSemaphore, zero for Pool-engine non-ES ops), but
    # Tile's wait assignment happily packs 2-3. Split the extras onto
    # single-wait NoOps (ES chunks for Pool) on the same engine.
    if not getattr(TileContext, "_ant_wait_split", False):
        _orig_commit = TileContext._commit_instruction

        def _commit_split(self, inst, lazy_reg_writes=True):
            si = inst.sync_info
            is_es = isinstance(inst, mybir.InstEventSemaphore)
            is_pool = inst.engine == mybir.EngineType.Pool
            limit = 2 if is_es else (0 if is_pool else 1)
            if si is not None and si.on_wait and len(si.on_wait) > limit:
                waits = list(si.on_wait)
                extras = waits[: len(waits) - limit]
                if is_pool:
                    for i in range(0, len(extras), 2):
                        es = mybir.InstEventSemaphore(
                            name=f"{inst.name}-sw{i}",
                            sync_info=mybir.SyncInfo(
                                on_wait=extras[i : i + 2], on_update=[]
                            ),
                            engine=inst.engine,
                        )
                        _orig_commit(self, es, lazy_reg_writes)
                else:
                    for i, w in enumerate(extras):
                        nop = mybir.InstNoOp(
                            name=f"{inst.name}-sw{i}",
                            sync_info=mybir.SyncInfo(on_wait=[w], on_update=[]),
                            bass_nofuse=True,
                            engine=inst.engine,
                        )
                        _orig_commit(self, nop, lazy_reg_writes)
                inst.sync_info = mybir.SyncInfo(
                    on_wait=waits[len(waits) - limit :], on_update=list(si.on_update)
                )
            return _orig_commit(self, inst, lazy_reg_writes)

        TileContext._commit_instruction = _commit_split
        TileContext._ant_wait_split = True

    # The stock TileContext tail drain packs every outstanding sem wait into
    # a single InstDrain; this walrus caps non-EventSemaphore instructions at
    # one wait. Emit one drain per outstanding semaphore instead.
    def _drain_and_barrier(self, tick_clock, wait_clock):
        nc = self.nc
        gc = tick_clock.global_clock
        n = len(gc)
        for i in range(n):
            t = gc[i]
            if t <= 0:
                continue
            vc = VectorClock([0] * n)
            vc.require_at_least(i, t)
            d = nc.sync.drain()
            wait_clock.add_sem_waits(d.ins, ScopedClock({None: vc}))
        nc.all_engine_barrier()
        assert self.sems is not None
        popped = nc._tile_sem_poison_stack.pop()
        assert popped is self._sem_poison
        nc.clear_and_free_semaphores(list(self.sems.allocated().values()))
        nc.all_engine_barrier()

    TileContext._drain_and_barrier = _drain_and_barrier


def _build_nc():
    import concourse.bass as bass
    import concourse.mybir as mybir
    from concourse.tile import TileContext

    _install_tile_patches()

    f32 = mybir.dt.float32
    bf16 = mybir.dt.bfloat16
    f8 = mybir.dt.float8e4
    Alu = mybir.AluOpType
    Act = mybir.ActivationFunctionType
    X = mybir.AxisListType

    nc = bass.Bass()
    # alpha planes: row g*128+p, free [o(7), t(8), nl(16), k(16)] bf16
    alpha_d = nc.declare_dram_parameter(
        "alpha_p", [NG * P, NPL * GF], bf16, isOutput=False
    )
    # beta pre-transposed on the host into chunk layout:
    # beta_t[el, t*2048 + c*128 + p] = beta[t*128 + p, c*126 + el]
    beta_d = nc.declare_dram_parameter("beta_t", [ECH, T * NCH * P], bf16, isOutput=False)
    theta_d = nc.declare_dram_parameter("theta_t", [P, T], f32, isOutput=False)
    mask_d = nc.declare_dram_parameter("mask_c", [P, NMASK * SW], bf16, isOutput=False)
    mt_d = nc.declare_dram_parameter("mt_c", [ECH, NCH * EDW], bf16, isOutput=False)
    out_d = nc.declare_dram_parameter("loss_part", [1, 1], f32, isOutput=True)

    with TileContext(nc) as tc:
        with (
            tc.tile_pool(name="consts", bufs=1) as cp,
            tc.tile_pool(name="planes", bufs=3) as plp,
            tc.tile_pool(name="tree1", bufs=1) as trp1,
            tc.tile_pool(name="tree2", bufs=2) as trp2,
            tc.tile_pool(name="persist", bufs=1) as pp,
            tc.tile_pool(name="smallp", bufs=2) as sp,
            tc.tile_pool(name="finp", bufs=1) as fp_,
            tc.tile_pool(name="betap", bufs=1) as bp,
            tc.tile_pool(name="ebtp", bufs=2) as ep,
            tc.tile_pool(name="psc", bufs=2, space="PSUM") as psc,
        ):
            # first plane-group DMA gates the DVE pipeline; split it so the
            # tree's first ops can start on the front half
            pl_tiles = [
                plp.tile([P, NPL * GF], bf16, tag="pl", name=f"pl{i}")
                for i in range(3)
            ]
            nc.sync.dma_start(
                pl_tiles[0][:, 0 : 2 * GF], alpha_d[0:P, 0 : 2 * GF]
            )
            nc.sync.dma_start(
                pl_tiles[0][:, 2 * GF : 4 * GF], alpha_d[0:P, 2 * GF : 4 * GF]
            )
            nc.sync.dma_start(
                pl_tiles[0][:, 4 * GF : NPL * GF], alpha_d[0:P, 4 * GF : NPL * GF]
            )

            mask_sb = cp.tile([P, NMASK * SW], bf16)
            nc.scalar.dma_start(mask_sb[:, :], mask_d[:, :])
            mt_sb = cp.tile([ECH, NCH * EDW], bf16)
            nc.scalar.dma_start(mt_sb[:, :], mt_d[:, :])
            theta_sb = cp.tile([P, T], f32)
            nc.scalar.dma_start(theta_sb[:, :], theta_d[:, :])
            ones_sb = cp.tile([P, 1], f32)
            nc.vector.memset(ones_sb[:, :], 1.0)

            # prefetch groups 1-2 + the first beta tiles
            nc.sync.dma_start(pl_tiles[1][:, :], alpha_d[P : 2 * P, :])
            nc.sync.dma_start(pl_tiles[2][:, :], alpha_d[2 * P : 3 * P, :])
            b_tiles = [
                bp.tile([ECH, NCH * P], bf16, tag=f"b{t}", name=f"bt{t}")
                for t in range(T)
            ]
            for t in range(T):
                nc.sync.dma_start(
                    b_tiles[t][:, :], beta_d[:, t * NCH * P : (t + 1) * NCH * P]
                )

            w_sb = pp.tile([P, NG * GF], bf16)    # softmax weights, grouped
            ed_sb = pp.tile([P, T * EDW], f32)    # DP state, zero-init
            tmp_sb = pp.tile([P, T * SW], f32)    # DP step scratch
            nc.vector.memset(ed_sb[:, :], 0.0)

            ed3 = ed_sb.rearrange("p (t k) -> p t k", t=T)
            tmp3 = tmp_sb.rearrange("p (t k) -> p t k", k=SW)

            c_ps = psc.tile([P, T * EDW], f32, tag="c", bufs=1)

            for g in range(NG):
                pl = pl_tiles[g % 3]
                pv = pl.rearrange("p (o f) -> p o f", o=NPL)

                # max tree over 7 planes, all operands dense bf16 (2x mode)
                mxg = trp2.tile([P, GF], bf16, tag="mx")
                if g == 0:
                    # chase the three g0 DMA pieces
                    lb = trp1.tile([P, GF], bf16, tag="lb")
                    nc.vector.tensor_tensor(lb[:, :], pv[:, 0, :], pv[:, 1, :], Alu.max)
                    la = trp1.tile([P, GF], bf16, tag="la")
                    nc.vector.tensor_tensor(la[:, :], pv[:, 2, :], pv[:, 3, :], Alu.max)
                    nc.vector.tensor_tensor(lb[:, :], lb[:, :], la[:, :], Alu.max)
                    lc = trp1.tile([P, GF], bf16, tag="lc")
                    nc.vector.tensor_tensor(lc[:, :], pv[:, 4, :], pv[:, 5, :], Alu.max)
                    nc.vector.tensor_tensor(lc[:, :], lc[:, :], pv[:, 6, :], Alu.max)
                    nc.vector.tensor_tensor(mxg[:, :], lb[:, :], lc[:, :], Alu.max)
                else:
                    # L1: max(planes 0-2, planes 3-5) in one dense op
                    la = trp1.tile([P, 3 * GF], bf16, tag="la")
                    l3 = la.rearrange("p (i f) -> p i f", i=3)
                    nc.vector.tensor_tensor(
                        l3[:, :, :], pv[:, 0:3, :], pv[:, 3:6, :], Alu.max
                    )
                    lb = trp1.tile([P, GF], bf16, tag="lb")
                    nc.vector.tensor_tensor(
                        lb[:, :], l3[:, 0, :], l3[:, 1, :], Alu.max
                    )
                    lc = trp1.tile([P, GF], bf16, tag="lc")
                    nc.vector.tensor_tensor(lc[:, :], l3[:, 2, :], pv[:, 6, :], Alu.max)
                    nc.vector.tensor_tensor(mxg[:, :], lb[:, :], lc[:, :], Alu.max)

                # reuse buffer 0 for group 3 once group 0's tree is done
                if g == 0:
                    nc.sync.dma_start(
                        pl_tiles[0][:, :], alpha_d[3 * P : 4 * P, :]
                    )

                # softmax numerator without max-subtraction (|logits| <~ 6)
                e_sl = w_sb[:, g * GF : (g + 1) * GF]
                nc.scalar.activation(e_sl, mxg[:, :], Act.Exp)
                if g == 0:
                    # zero the invalid window slots of nodes 0..13
                    e30 = e_sl.rearrange("p (t m) -> p t m", m=GW)[
                        :, :, 0 : NMASK * SW
                    ]
                    mask_b = mask_sb.rearrange(
                        "p (o m) -> p o m", o=1
                    ).broadcast_to((P, T, NMASK * SW))
                    nc.vector.tensor_mul(e30, e30, mask_b)

                s_g = sp.tile([P, T * GN], f32, tag="s")
                nc.vector.reduce_sum(
                    s_g[:, :], e_sl.rearrange("p (n k) -> p n k", k=SW), axis=X.X
                )
                lns = sp.tile([P, T * GN], f32, tag="lns")
                nc.scalar.activation(lns[:, :], s_g[:, :], Act.Ln)
                # fused exp(-ln s) + broadcast-expand to [., n, 16] on ACT so
                # the normalize multiply below runs dense bf16 at 2x
                rse = sp.tile([P, GF], bf16, tag="rse")
                nc.scalar.activation(
                    rse.rearrange("p (n k) -> p n k", k=SW),
                    lns.rearrange("p (n o) -> p n o", o=1).broadcast_to(
                        (P, T * GN, SW)
                    ),
                    Act.Exp,
                    scale=-1.0,
                )
                nc.vector.tensor_mul(e_sl, e_sl, rse[:, :])

                # DP steps for this group's nodes (all 8 stage slots at once)
                wg = w_sb[:, g * GF : (g + 1) * GF].rearrange(
                    "p (t n k) -> p t n k", t=T, k=SW
                )
                for nl in range(GN):
                    j = g * GN + nl + 2
                    wid = min(j, SW)
                    base = j - wid
                    nc.vector.scalar_tensor_tensor(
                        tmp3[:, :, 0:wid],
                        ed3[:, :, base : base + wid],
                        1.0,
                        wg[:, :, nl, 0:wid],
                        Alu.add,
                        Alu.mult,
                    )
                    nc.vector.reduce_sum(
                        ed3[:, :, j : j + 1], tmp3[:, :, 0:wid], axis=X.X
                    )

            # ---- beta phase (low priority: fills scalar/PE gaps) ----
            for t in range(T):
                eb_t = ep.tile([ECH, NCH * P], bf16, tag="eb")
                half = NCH * P // 2
                nc.scalar.activation(
                    eb_t[:, 0:half], b_tiles[t][:, 0:half], Act.Exp
                )
                nc.scalar.activation(
                    eb_t[:, half:], b_tiles[t][:, half:], Act.Exp
                )
                for c in range(NCH):
                    nc.tensor.matmul(
                        c_ps[:, t * EDW : (t + 1) * EDW],
                        eb_t[:, c * P : (c + 1) * P],
                        mt_sb[:, c * EDW : (c + 1) * EDW],
                        start=(c == 0),
                        stop=(c == NCH - 1),
                    )

            # ---- final dots: batched over all 8 stage slots ----
            prod = fp_.tile([P, T * (EDW - 1)], f32, tag="prod")
            q = fp_.tile([P, T], f32, tag="q")
            cv = c_ps.rearrange("p (t k) -> p t k", k=EDW)
            nc.vector.scalar_tensor_tensor(
                prod.rearrange("p (t k) -> p t k", k=EDW - 1),
                ed3[:, :, 0 : EDW - 1],
                0.0,
                cv[:, :, 0 : EDW - 1],
                Alu.add,
                Alu.mult,
            )
            nc.vector.reduce_sum(
                q.rearrange("p (t k) -> p t k", k=1),
                prod.rearrange("p (t k) -> p t k", k=EDW - 1),
                axis=X.X,
            )
            rsb = fp_.tile([P, T], f32, tag="rsb")
            nc.vector.reciprocal(rsb[:, :], cv[:, :, EDW - 1])
            rst = fp_.tile([P, T], f32, tag="rst")
            nc.vector.tensor_mul(rst[:, :], rsb[:, :], theta_sb[:, :])
            acc = fp_.tile([P, T], f32, tag="acc")
            nc.vector.tensor_mul(acc[:, :], q[:, :], rst[:, :])

            # ---- final reduction: 8 cols then 128 partitions ----
            accsum = fp_.tile([P, 1], f32, tag="accsum")
            nc.vector.reduce_sum(accsum[:, :], acc[:, :], axis=X.X)
            out_ps = psc.tile([1, 1], f32, tag="outp", bufs=1)
            nc.tensor.matmul(
                out_ps[:, :], accsum[:, :], ones_sb[:, :], start=True, stop=True
            )
            out_sb = fp_.tile([1, 1], f32, tag="outs")
            nc.scalar.copy(out_sb[:, :], out_ps[:, :])
            nc.sync.dma_start(out_d[:, :], out_sb[:, :])

    return nc


def _get_compiled():
    if "nc" not in _CACHE:
        _CACHE["nc"] = _build_nc()
        _CACHE["consts"] = _host_consts()
    return _CACHE["nc"], _CACHE["consts"]


def _in_maps(alpha, beta, theta):
    import ml_dtypes

    mt, mask = _get_compiled()[1]
    alpha = np.ascontiguousarray(alpha, dtype=np.float32)
    beta = np.ascontiguousarray(beta, dtype=np.float32)
    theta = np.ascontiguousarray(theta, dtype=np.float32)
    alpha_bf = alpha.astype(ml_dtypes.bfloat16)
    beta_bf = beta.astype(ml_dtypes.bfloat16)
    maps = []
    for c in range(N_CORES):
        sl = slice(c * S_CORE, (c + 1) * S_CORE)
        # [t, p, g, nl, k, o] -> [g, p, o, t, nl, k], drop op 7
        A = alpha_bf[sl].reshape(T, P, NG, GN, SW, 8)
        planes = np.ascontiguousarray(A.transpose(2, 1, 5, 0, 3, 4)[:, :, :NPL])
        # [el, t*2048 + ch*128 + p] = beta[t*128 + p, ch*126 + el]
        beta_t = np.ascontiguousarray(
            beta_bf[sl].reshape(T, P, NCH, ECH).transpose(3, 0, 2, 1).reshape(ECH, -1)
        )
        maps.append(
            {
                "alpha_p": planes.reshape(NG * P, NPL * GF),
                "beta_t": beta_t,
                "theta_t": np.ascontiguousarray(theta[sl].reshape(T, P).T),
                "mask_c": mask,
                "mt_c": mt,
            }
        )
    return maps


def _run(alpha, beta, theta, **spmd_kwargs):
    from concourse.bass_utils import run_bass_kernel_spmd

    nc, _ = _get_compiled()
    res = run_bass_kernel_spmd(
        nc, _in_maps(alpha, beta, theta), core_ids=list(range(N_CORES)), **spmd_kwargs
    )
    total = np.float32(0.0)
    for r in res.results:
        total += np.float32(r["loss_part"][0, 0])
    return np.float32(total), res


def kernel(alpha, beta, theta):
    out, _ = _run(alpha, beta, theta)
    return out


# revision 11
# speedup vs baseline: 1.5369x; 1.0113x over previous
"""Trainium2 Bass kernel for the Expected-Depth DP loss.

Computation (see reference):
  - edge_max = max over first 7 of 8 op-logits          [S, 64, 16]
  - w        = masked softmax over the 16-wide window   [S, 64, 16]
  - DP scan:  ed[j] = sum_k w[j,k] * (ed[base+k] + 1),  j = 2..65
  - loss     = sum_s theta[s] * softmax(beta[s]) . (ed[ii] + ed[jj])

Sharding: S=8192 stages split across 8 cores (pure data parallel,
1024 stages/core as 128 partitions x 8 free slots). Per-core partial
losses are summed on the host.

v2 layout/engine choices:
  - alpha staged in HBM as 7 op-major bf16 planes (op 7 unused),
    node-grouped, streamed on the sync HWDGE ring (SWDGE cast-DMA from
    fp8 measured ~2x slower and its descriptor rings degrade DVE 2x).
  - max-of-7 as a 4-instruction bf16 tensor_tensor max tree (2x mode)
    instead of a 1x tensor_reduce.
  - per-node-group pipeline: tree/exp/softmax/DP for nodes [16g,16g+16)
    overlap the next group's plane DMA.
  - the softmax reciprocal is broadcast-expanded on the scalar engine so
    the normalize multiply runs dense bf16 at DVE 2x.
  - beta rides the scalar-engine HWDGE ring in bf16; its exps/matmuls
    are emitted after the group loop so they fill scalar-engine gaps.
"""

import numpy as np

SW = 16          # DP window
NN = 64          # nodes per stage
S = 8192         # stages
E = 2016         # beta edges
P = 128          # SBUF partitions
N_CORES = 8
S_CORE = S // N_CORES        # 1024
T = S_CORE // P              # 8 stage slots per partition
NG = 4                       # node groups
GN = NN // NG                # 16 nodes per group
GW = GN * SW                 # 256 edge_max floats per stage per group
GF = T * GW                  # 2048 free elems per group tile
NPL = 7                      # op planes
EDW = 67                     # ed row stride (66 node slots + 1 pad)
NCH = 16                     # beta column chunks
ECH = E // NCH               # 126 edges per chunk
NMASK = 14                   # nodes with partially-valid windows

_CACHE = {}


def _host_consts():
    import ml_dtypes

    ii, jj = [], []
    for i in range(2, NN + 1):
        for j in range(i + 1, NN + 2):
            ii.append(i)
            jj.append(j)
    ii = np.asarray(ii)
    jj = np.asarray(jj)
    # incidence matrix chunks: mt[e_local, c*67 + k] = [ii==k] + [jj==k],
    # column 66 of each chunk is all ones (softmax denominator)
    mt = np.zeros((NCH, ECH, EDW), np.float32)
    for e in range(E):
        c, el = divmod(e, ECH)
        mt[c, el, ii[e]] += 1.0
        mt[c, el, jj[e]] += 1.0
        mt[c, el, EDW - 1] = 1.0
    mt = np.ascontiguousarray(
        mt.transpose(1, 0, 2).reshape(ECH, NCH * EDW)
    ).astype(ml_dtypes.bfloat16)
    # validity mask for the first 14 nodes (node n: rows k < n+2 valid)
    mask = np.zeros((NMASK, SW), np.float32)
    for n in range(NMASK):
        mask[n, : n + 2] = 1.0
    mask = np.ascontiguousarray(
        np.broadcast_to(mask.reshape(1, NMASK * SW), (P, NMASK * SW))
    ).astype(ml_dtypes.bfloat16)
    return mt, mask


def _install_tile_patches():
    import concourse.mybir as mybir
    from concourse.tile import TileContext
    from concourse.vector_clock import ScopedClock, VectorClock

    # This walrus build rejects TPB instructions carrying more than one sem
    # wait (two for EventSemaphore, zero for Pool-engine non-ES ops), but
    # Tile's wait assignment happily packs 2-3. Split the extras onto
    # single-wait NoOps (ES chunks for Pool) on the same engine.
    if not getattr(TileContext, "_ant_wait_split", False):
        _orig_commit = TileContext._commit_instruction

        def _commit_split(self, inst, lazy_reg_writes=True):
            si = inst.sync_info
            is_es = isinstance(inst, mybir.InstEventSemaphore)
            is_pool = inst.engine == mybir.EngineType.Pool
            limit = 2 if is_es else (0 if is_pool else 1)
            if si is not None and si.on_wait and len(si.on_wait) > limit:
                waits = list(si.on_wait)
                extras = waits[: len(waits) - limit]
                if is_pool:
                    for i in range(0, len(extras), 2):
                        es = mybir.InstEventSemaphore(
                            name=f"{inst.name}-sw{i}",
                            sync_info=mybir.SyncInfo(
                                on_wait=extras[i : i + 2], on_update=[]
                            ),
                            engine=inst.engine,
                        )
                        _orig_commit(self, es, lazy_reg_writes)
                else:
                    for i, w in enumerate(extras):
                        nop = mybir.InstNoOp(
                            name=f"{inst.name}-sw{i}",
                            sync_info=mybir.SyncInfo(on_wait=[w], on_update=[]),
                            bass_nofuse=True,
                            engine=inst.engine,
                        )
                        _orig_commit(self, nop, lazy_reg_writes)
                inst.sync_info = mybir.SyncInfo(
                    on_wait=waits[len(waits) - limit :], on_update=list(si.on_update)
                )
            return _orig_commit(self, inst, lazy_reg_writes)

        TileContext._commit_instruction = _commit_split
        TileContext._ant_wait_split = True

    # The stock TileContext tail drain packs every outstanding sem wait into
    # a single InstDrain; this walrus caps non-EventSemaphore instructions at
    # one wait. Emit one drain per outstanding semaphore instead.
    def _drain_and_barrier(self, tick_clock, wait_clock):
        nc = self.nc
        gc = tick_clock.global_clock
        n = len(gc)
        for i in range(n):
            t = gc[i]
            if t <= 0:
                continue
            vc = VectorClock([0] * n)
            vc.require_at_least(i, t)
            d = nc.sync.drain()
            wait_clock.add_sem_waits(d.ins, ScopedClock({None: vc}))
        nc.all_engine_barrier()
        assert self.sems is not None
        popped = nc._tile_sem_poison_stack.pop()
        assert popped is self._sem_poison
        nc.clear_and_free_semaphores(list(self.sems.allocated().values()))
        nc.all_engine_barrier()

    TileContext._drain_and_barrier = _drain_and_barrier


def _build_nc():
    import concourse.bass as bass
    import concourse.mybir as mybir
    from concourse.tile import TileContext

    _install_tile_patches()

    f32 = mybir.dt.float32
    bf16 = mybir.dt.bfloat16
    f8 = mybir.dt.float8e4
    Alu = mybir.AluOpType
    Act = mybir.ActivationFunctionType
    X = mybir.AxisListType

    nc = bass.Bass()
    # alpha planes: row g*128+p, free [o(7), t(8), nl(16), k(16)] bf16
    alpha_d = nc.declare_dram_parameter(
        "alpha_p", [NG * P, NPL * GF], bf16, isOutput=False
    )
    # beta pre-transposed on the host into chunk layout:
    # beta_t[el, t*2048 + c*128 + p] = beta[t*128 + p, c*126 + el]
    beta_d = nc.declare_dram_parameter("beta_t", [ECH, T * NCH * P], bf16, isOutput=False)
    theta_d = nc.declare_dram_parameter("theta_t", [P, T], f32, isOutput=False)
    mask_d = nc.declare_dram_parameter("mask_c", [P, NMASK * SW], bf16, isOutput=False)
    mt_d = nc.declare_dram_parameter("mt_c", [ECH, NCH * EDW], bf16, isOutput=False)
    out_d = nc.declare_dram_parameter("loss_part", [1, 1], f32, isOutput=True)

    with TileContext(nc) as tc:
        with (
            tc.tile_pool(name="consts", bufs=1) as cp,
            tc.tile_pool(name="planes", bufs=3) as plp,
            tc.tile_pool(name="tree1", bufs=1) as trp1,
            tc.tile_pool(name="tree2", bufs=2) as trp2,
            tc.tile_pool(name="persist", bufs=1) as pp,
            tc.tile_pool(name="smallp", bufs=2) as sp,
            tc.tile_pool(name="finp", bufs=1) as fp_,
            tc.tile_pool(name="betap", bufs=1) as bp,
            tc.tile_pool(name="ebtp", bufs=2) as ep,
            tc.tile_pool(name="psc", bufs=2, space="PSUM") as psc,
        ):
            # first plane-group DMA gates the DVE pipeline; split it so the
            # tree's first ops can start on the front half
            pl_tiles = [
                plp.tile([P, NPL * GF], bf16, tag="pl", name=f"pl{i}")
                for i in range(3)
            ]
            nc.sync.dma_start(
                pl_tiles[0][:, 0 : 2 * GF], alpha_d[0:P, 0 : 2 * GF]
            )
            nc.sync.dma_start(
                pl_tiles[0][:, 2 * GF : 4 * GF], alpha_d[0:P, 2 * GF : 4 * GF]
            )
            nc.sync.dma_start(
                pl_tiles[0][:, 4 * GF : NPL * GF], alpha_d[0:P, 4 * GF : NPL * GF]
            )

            mask_sb = cp.tile([P, NMASK * SW], bf16)
            nc.scalar.dma_start(mask_sb[:, :], mask_d[:, :])
            mt_sb = cp.tile([ECH, NCH * EDW], bf16)
            nc.scalar.dma_start(mt_sb[:, :], mt_d[:, :])
            theta_sb = cp.tile([P, T], f32)
            nc.scalar.dma_start(theta_sb[:, :], theta_d[:, :])
            ones_sb = cp.tile([P, 1], f32)
            nc.vector.memset(ones_sb[:, :], 1.0)

            # prefetch groups 1-2 + the first beta tiles
            nc.sync.dma_start(pl_tiles[1][:, :], alpha_d[P : 2 * P, :])
            nc.sync.dma_start(pl_tiles[2][:, :], alpha_d[2 * P : 3 * P, :])
            b_tiles = [
                bp.tile([ECH, NCH * P], bf16, tag=f"b{t}", name=f"bt{t}")
                for t in range(T)
            ]
            for t in range(T):
                nc.sync.dma_start(
                    b_tiles[t][:, :], beta_d[:, t * NCH * P : (t + 1) * NCH * P]
                )

            w_sb = pp.tile([P, NG * GF], bf16)    # softmax weights, grouped
            ed_sb = pp.tile([P, T * EDW], f32)    # DP state, zero-init
            tmp_sb = pp.tile([P, T * SW], f32)    # DP step scratch
            nc.vector.memset(ed_sb[:, :], 0.0)

            ed3 = ed_sb.rearrange("p (t k) -> p t k", t=T)
            tmp3 = tmp_sb.rearrange("p (t k) -> p t k", k=SW)

            c_ps = psc.tile([P, T * EDW], f32, tag="c", bufs=1)

            for g in range(NG):
                pl = pl_tiles[g % 3]
                pv = pl.rearrange("p (o f) -> p o f", o=NPL)

                # max tree over 7 planes, all operands dense bf16 (2x mode)
                mxg = trp2.tile([P, GF], bf16, tag="mx")
                if g == 0:
                    # chase the three g0 DMA pieces
                    lb = trp1.tile([P, GF], bf16, tag="lb")
                    nc.vector.tensor_tensor(lb[:, :], pv[:, 0, :], pv[:, 1, :], Alu.max)
                    la = trp1.tile([P, GF], bf16, tag="la")
                    nc.vector.tensor_tensor(la[:, :], pv[:, 2, :], pv[:, 3, :], Alu.max)
                    nc.vector.tensor_tensor(lb[:, :], lb[:, :], la[:, :], Alu.max)
                    lc = trp1.tile([P, GF], bf16, tag="lc")
                    nc.vector.tensor_tensor(lc[:, :], pv[:, 4, :], pv[:, 5, :], Alu.max)
                    nc.vector.tensor_tensor(lc[:, :], lc[:, :], pv[:, 6, :], Alu.max)
                    nc.vector.tensor_tensor(mxg[:, :], lb[:, :], lc[:, :], Alu.max)
                else:
                    # L1: max(planes 0-2, planes 3-5) in one dense op
                    la = trp1.tile([P, 3 * GF], bf16, tag="la")
                    l3 = la.rearrange("p (i f) -> p i f", i=3)
                    nc.vector.tensor_tensor(
                        l3[:, :, :], pv[:, 0:3, :], pv[:, 3:6, :], Alu.max
                    )
                    lb = trp1.tile([P, GF], bf16, tag="lb")
                    nc.vector.tensor_tensor(
                        lb[:, :], l3[:, 0, :], l3[:, 1, :], Alu.max
                    )
                    lc = trp1.tile([P, GF], bf16, tag="lc")
                    nc.vector.tensor_tensor(lc[:, :], l3[:, 2, :], pv[:, 6, :], Alu.max)
                    nc.vector.tensor_tensor(mxg[:, :], lb[:, :], lc[:, :], Alu.max)

                # reuse buffer 0 for group 3 once group 0's tree is done
                if g == 0:
                    nc.sync.dma_start(
                        pl_tiles[0][:, :], alpha_d[3 * P : 4 * P, :]
                    )

                # softmax numerator without max-subtraction (|logits| <~ 6)
                e_sl = w_sb[:, g * GF : (g + 1) * GF]
                nc.scalar.activation(e_sl, mxg[:, :], Act.Exp)
                if g == 0:
                    # zero the invalid window slots of nodes 0..13
                    e30 = e_sl.rearrange("p (t m) -> p t m", m=GW)[
                        :, :, 0 : NMASK * SW
                    ]
                    mask_b = mask_sb.rearrange(
                        "p (o m) -> p o m", o=1
                    ).broadcast_to((P, T, NMASK * SW))
                    nc.vector.tensor_mul(e30, e30, mask_b)

                # two bf16 pair-add levels at 2x, then a 4-wide 1x reduce
                ph = sp.tile([P, T * GN * 8], bf16, tag="ph")
                e4 = e_sl.rearrange("p (n k) -> p n k", k=SW)
                nc.vector.tensor_add(
                    ph.rearrange("p (n k) -> p n k", k=8),
                    e4[:, :, 0:8],
                    e4[:, :, 8:16],
                )
                pq = sp.tile([P, T * GN * 4], bf16, tag="pq")
                p8 = ph.rearrange("p (n k) -> p n k", k=8)
                nc.vector.tensor_add(
                    pq.rearrange("p (n k) -> p n k", k=4),
                    p8[:, :, 0:4],
                    p8[:, :, 4:8],
                )
                s_g = sp.tile([P, T * GN], f32, tag="s")
                nc.vector.reduce_sum(
                    s_g[:, :], pq.rearrange("p (n k) -> p n k", k=4), axis=X.X
                )
                lns = sp.tile([P, T * GN], f32, tag="lns")
                nc.scalar.activation(lns[:, :], s_g[:, :], Act.Ln)
                # fused exp(-ln s) + broadcast-expand to [., n, 16] on ACT so
                # the normalize multiply below runs dense bf16 at 2x
                rse = sp.tile([P, GF], bf16, tag="rse")
                nc.scalar.activation(
                    rse.rearrange("p (n k) -> p n k", k=SW),
                    lns.rearrange("p (n o) -> p n o", o=1).broadcast_to(
                        (P, T * GN, SW)
                    ),
                    Act.Exp,
                    scale=-1.0,
                )
                nc.vector.tensor_mul(e_sl, e_sl, rse[:, :])

                # DP steps for this group's nodes (all 8 stage slots at once)
                wg = w_sb[:, g * GF : (g + 1) * GF].rearrange(
                    "p (t n k) -> p t n k", t=T, k=SW
                )
                for nl in range(GN):
                    j = g * GN + nl + 2
                    wid = min(j, SW)
                    base = j - wid
                    nc.vector.scalar_tensor_tensor(
                        tmp3[:, :, 0:wid],
                        ed3[:, :, base : base + wid],
                        1.0,
                        wg[:, :, nl, 0:wid],
                        Alu.add,
                        Alu.mult,
                    )
                    nc.vector.reduce_sum(
                        ed3[:, :, j : j + 1], tmp3[:, :, 0:wid], axis=X.X
                    )

            # ---- beta phase (low priority: fills scalar/PE gaps) ----
            for t in range(T):
                eb_t = ep.tile([ECH, NCH * P], bf16, tag="eb")
                half = NCH * P // 2
                nc.scalar.activation(
                    eb_t[:, 0:half], b_tiles[t][:, 0:half], Act.Exp
                )
                nc.scalar.activation(
                    eb_t[:, half:], b_tiles[t][:, half:], Act.Exp
                )
                for c in range(NCH):
                    nc.tensor.matmul(
                        c_ps[:, t * EDW : (t + 1) * EDW],
                        eb_t[:, c * P : (c + 1) * P],
                        mt_sb[:, c * EDW : (c + 1) * EDW],
                        start=(c == 0),
                        stop=(c == NCH - 1),
                    )

            # ---- final dots: batched over all 8 stage slots ----
            prod = fp_.tile([P, T * (EDW - 1)], f32, tag="prod")
            q = fp_.tile([P, T], f32, tag="q")
            cv = c_ps.rearrange("p (t k) -> p t k", k=EDW)
            nc.vector.scalar_tensor_tensor(
                prod.rearrange("p (t k) -> p t k", k=EDW - 1),
                ed3[:, :, 0 : EDW - 1],
                0.0,
                cv[:, :, 0 : EDW - 1],
                Alu.add,
                Alu.mult,
            )
            nc.vector.reduce_sum(
                q.rearrange("p (t k) -> p t k", k=1),
                prod.rearrange("p (t k) -> p t k", k=EDW - 1),
                axis=X.X,
            )
            rsb = fp_.tile([P, T], f32, tag="rsb")
            nc.vector.reciprocal(rsb[:, :], cv[:, :, EDW - 1])
            rst = fp_.tile([P, T], f32, tag="rst")
            nc.vector.tensor_mul(rst[:, :], rsb[:, :], theta_sb[:, :])
            acc = fp_.tile([P, T], f32, tag="acc")
            nc.vector.tensor_mul(acc[:, :], q[:, :], rst[:, :])

            # ---- final reduction: 8 cols then 128 partitions ----
            accsum = fp_.tile([P, 1], f32, tag="accsum")
            nc.vector.reduce_sum(accsum[:, :], acc[:, :], axis=X.X)
            out_ps = psc.tile([1, 1], f32, tag="outp", bufs=1)
            nc.tensor.matmul(
                out_ps[:, :], accsum[:, :], ones_sb[:, :], start=True, stop=True
            )
            out_sb = fp_.tile([1, 1], f32, tag="outs")
            nc.scalar.copy(out_sb[:, :], out_ps[:, :])
            nc.sync.dma_start(out_d[:, :], out_sb[:, :])

    return nc


def _get_compiled():
    if "nc" not in _CACHE:
        _CACHE["nc"] = _build_nc()
        _CACHE["consts"] = _host_consts()
    return _CACHE["nc"], _CACHE["consts"]


def _in_maps(alpha, beta, theta):
    import ml_dtypes

    mt, mask = _get_compiled()[1]
    alpha = np.ascontiguousarray(alpha, dtype=np.float32)
    beta = np.ascontiguousarray(beta, dtype=np.float32)
    theta = np.ascontiguousarray(theta, dtype=np.float32)
    alpha_bf = alpha.astype(ml_dtypes.bfloat16)
    beta_bf = beta.astype(ml_dtypes.bfloat16)
    maps = []
    for c in range(N_CORES):
        sl = slice(c * S_CORE, (c + 1) * S_CORE)
        # [t, p, g, nl, k, o] -> [g, p, o, t, nl, k], drop op 7
        A = alpha_bf[sl].reshape(T, P, NG, GN, SW, 8)
        planes = np.ascontiguousarray(A.transpose(2, 1, 5, 0, 3, 4)[:, :, :NPL])
        # [el, t*2048 + ch*128 + p] = beta[t*128 + p, ch*126 + el]
        beta_t = np.ascontiguousarray(
            beta_bf[sl].reshape(T, P, NCH, ECH).transpose(3, 0, 2, 1).reshape(ECH, -1)
        )
        maps.append(
            {
                "alpha_p": planes.reshape(NG * P, NPL * GF),
                "beta_t": beta_t,
                "theta_t": np.ascontiguousarray(theta[sl].reshape(T, P).T),
                "mask_c": mask,
                "mt_c": mt,
            }
        )
    return maps


def _run(alpha, beta, theta, **spmd_kwargs):
    from concourse.bass_utils import run_bass_kernel_spmd

    nc, _ = _get_compiled()
    res = run_bass_kernel_spmd(
        nc, _in_maps(alpha, beta, theta), core_ids=list(range(N_CORES)), **spmd_kwargs
    )
    total = np.float32(0.0)
    for r in res.results:
        total += np.float32(r["loss_part"][0, 0])
    return np.float32(total), res


def kernel(alpha, beta, theta):
    out, _ = _run(alpha, beta, theta)
    return out


# revision 12
# speedup vs baseline: 1.6066x; 1.0454x over previous
"""Trainium2 Bass kernel for the Expected-Depth DP loss.

Computation (see reference):
  - edge_max = max over first 7 of 8 op-logits          [S, 64, 16]
  - w        = masked softmax over the 16-wide window   [S, 64, 16]
  - DP scan:  ed[j] = sum_k w[j,k] * (ed[base+k] + 1),  j = 2..65
  - loss     = sum_s theta[s] * softmax(beta[s]) . (ed[ii] + ed[jj])

Sharding: S=8192 stages split across 8 cores (pure data parallel,
1024 stages/core as 128 partitions x 8 free slots). Per-core partial
losses are summed on the host.

v2 layout/engine choices:
  - alpha staged in HBM as 7 op-major bf16 planes (op 7 unused),
    node-grouped, streamed on the sync HWDGE ring (SWDGE cast-DMA from
    fp8 measured ~2x slower and its descriptor rings degrade DVE 2x).
  - max-of-7 as a 4-instruction bf16 tensor_tensor max tree (2x mode)
    instead of a 1x tensor_reduce.
  - per-node-group pipeline: tree/exp/softmax/DP for nodes [16g,16g+16)
    overlap the next group's plane DMA.
  - the softmax reciprocal is broadcast-expanded on the scalar engine so
    the normalize multiply runs dense bf16 at DVE 2x.
  - beta rides the scalar-engine HWDGE ring in bf16; its exps/matmuls
    are emitted after the group loop so they fill scalar-engine gaps.
"""

import numpy as np

SW = 16          # DP window
NN = 64          # nodes per stage
S = 8192         # stages
E = 2016         # beta edges
P = 128          # SBUF partitions
N_CORES = 8
S_CORE = S // N_CORES        # 1024
T = S_CORE // P              # 8 stage slots per partition
NG = 4                       # node groups
GN = NN // NG                # 16 nodes per group
GW = GN * SW                 # 256 edge_max floats per stage per group
GF = T * GW                  # 2048 free elems per group tile
NPL = 7                      # op planes
EDW = 67                     # ed row stride (66 node slots + 1 pad)
NCH = 16                     # beta column chunks
ECH = E // NCH               # 126 edges per chunk
NMASK = 14                   # nodes with partially-valid windows

_CACHE = {}


def _host_consts():
    import ml_dtypes

    ii, jj = [], []
    for i in range(2, NN + 1):
        for j in range(i + 1, NN + 2):
            ii.append(i)
            jj.append(j)
    ii = np.asarray(ii)
    jj = np.asarray(jj)
    # incidence matrix chunks: mt[e_local, c*67 + k] = [ii==k] + [jj==k],
    # column 66 of each chunk is all ones (softmax denominator)
    mt = np.zeros((NCH, ECH, EDW), np.float32)
    for e in range(E):
        c, el = divmod(e, ECH)
        mt[c, el, ii[e]] += 1.0
        mt[c, el, jj[e]] += 1.0
        mt[c, el, EDW - 1] = 1.0
    mt = np.ascontiguousarray(
        mt.transpose(1, 0, 2).reshape(ECH, NCH * EDW)
    ).astype(ml_dtypes.bfloat16)
    # validity mask for the first 14 nodes (node n: rows k < n+2 valid)
    mask = np.zeros((NMASK, SW), np.float32)
    for n in range(NMASK):
        mask[n, : n + 2] = 1.0
    mask = np.ascontiguousarray(
        np.broadcast_to(mask.reshape(1, NMASK * SW), (P, NMASK * SW))
    ).astype(ml_dtypes.bfloat16)
    return mt, mask


def _install_tile_patches():
    import concourse.mybir as mybir
    from concourse.tile import TileContext
    from concourse.vector_clock import ScopedClock, VectorClock

    # This walrus build rejects TPB instructions carrying more than one sem
    # wait (two for EventSemaphore, zero for Pool-engine non-ES ops), but
    # Tile's wait assignment happily packs 2-3. Split the extras onto
    # single-wait NoOps (ES chunks for Pool) on the same engine.
    if not getattr(TileContext, "_ant_wait_split", False):
        _orig_commit = TileContext._commit_instruction

        def _commit_split(self, inst, lazy_reg_writes=True):
            si = inst.sync_info
            is_es = isinstance(inst, mybir.InstEventSemaphore)
            is_pool = inst.engine == mybir.EngineType.Pool
            limit = 2 if is_es else (0 if is_pool else 1)
            if si is not None and si.on_wait and len(si.on_wait) > limit:
                waits = list(si.on_wait)
                extras = waits[: len(waits) - limit]
                if is_pool:
                    for i in range(0, len(extras), 2):
                        es = mybir.InstEventSemaphore(
                            name=f"{inst.name}-sw{i}",
                            sync_info=mybir.SyncInfo(
                                on_wait=extras[i : i + 2], on_update=[]
                            ),
                            engine=inst.engine,
                        )
                        _orig_commit(self, es, lazy_reg_writes)
                else:
                    for i, w in enumerate(extras):
                        nop = mybir.InstNoOp(
                            name=f"{inst.name}-sw{i}",
                            sync_info=mybir.SyncInfo(on_wait=[w], on_update=[]),
                            bass_nofuse=True,
                            engine=inst.engine,
                        )
                        _orig_commit(self, nop, lazy_reg_writes)
                inst.sync_info = mybir.SyncInfo(
                    on_wait=waits[len(waits) - limit :], on_update=list(si.on_update)
                )
            return _orig_commit(self, inst, lazy_reg_writes)

        TileContext._commit_instruction = _commit_split
        TileContext._ant_wait_split = True

    # The stock TileContext tail drain packs every outstanding sem wait into
    # a single InstDrain; this walrus caps non-EventSemaphore instructions at
    # one wait. Emit one drain per outstanding semaphore instead.
    def _drain_and_barrier(self, tick_clock, wait_clock):
        nc = self.nc
        gc = tick_clock.global_clock
        n = len(gc)
        for i in range(n):
            t = gc[i]
            if t <= 0:
                continue
            vc = VectorClock([0] * n)
            vc.require_at_least(i, t)
            d = nc.sync.drain()
            wait_clock.add_sem_waits(d.ins, ScopedClock({None: vc}))
        nc.all_engine_barrier()
        assert self.sems is not None
        popped = nc._tile_sem_poison_stack.pop()
        assert popped is self._sem_poison
        nc.clear_and_free_semaphores(list(self.sems.allocated().values()))
        nc.all_engine_barrier()

    TileContext._drain_and_barrier = _drain_and_barrier


def _build_nc():
    import concourse.bass as bass
    import concourse.mybir as mybir
    from concourse.tile import TileContext

    _install_tile_patches()

    f32 = mybir.dt.float32
    bf16 = mybir.dt.bfloat16
    f8 = mybir.dt.float8e4
    Alu = mybir.AluOpType
    Act = mybir.ActivationFunctionType
    X = mybir.AxisListType

    nc = bass.Bass()
    # alpha planes: row g*128+p, free [o(7), t(8), nl(16), k(16)] bf16
    alpha_d = nc.declare_dram_parameter(
        "alpha_p", [NG * P, NPL * GF], bf16, isOutput=False
    )
    # beta pre-transposed on the host into chunk layout:
    # beta_t[el, t*2048 + c*128 + p] = beta[t*128 + p, c*126 + el]
    beta_d = nc.declare_dram_parameter("beta_t", [ECH, T * NCH * P], bf16, isOutput=False)
    theta_d = nc.declare_dram_parameter("theta_t", [P, T], f32, isOutput=False)
    mt_d = nc.declare_dram_parameter("mt_c", [ECH, NCH * EDW], bf16, isOutput=False)
    out_d = nc.declare_dram_parameter("loss_part", [1, 1], f32, isOutput=True)

    with TileContext(nc) as tc:
        with (
            tc.tile_pool(name="consts", bufs=1) as cp,
            tc.tile_pool(name="planes", bufs=3) as plp,
            tc.tile_pool(name="tree1", bufs=1) as trp1,
            tc.tile_pool(name="tree2", bufs=2) as trp2,
            tc.tile_pool(name="persist", bufs=1) as pp,
            tc.tile_pool(name="smallp", bufs=2) as sp,
            tc.tile_pool(name="finp", bufs=1) as fp_,
            tc.tile_pool(name="betap", bufs=1) as bp,
            tc.tile_pool(name="ebtp", bufs=2) as ep,
            tc.tile_pool(name="psc", bufs=2, space="PSUM") as psc,
        ):
            # first plane-group DMA gates the DVE pipeline; split it so the
            # tree's first ops can start on the front half
            pl_tiles = [
                plp.tile([P, NPL * GF], bf16, tag="pl", name=f"pl{i}")
                for i in range(3)
            ]
            nc.sync.dma_start(
                pl_tiles[0][:, 0 : 2 * GF], alpha_d[0:P, 0 : 2 * GF]
            )
            nc.sync.dma_start(
                pl_tiles[0][:, 2 * GF : 4 * GF], alpha_d[0:P, 2 * GF : 4 * GF]
            )
            nc.sync.dma_start(
                pl_tiles[0][:, 4 * GF : NPL * GF], alpha_d[0:P, 4 * GF : NPL * GF]
            )

            mt_sb = cp.tile([ECH, NCH * EDW], bf16)
            nc.scalar.dma_start(mt_sb[:, :], mt_d[:, :])
            theta_sb = cp.tile([P, T], f32)
            nc.scalar.dma_start(theta_sb[:, :], theta_d[:, :])
            ones_sb = cp.tile([P, 1], f32)
            nc.vector.memset(ones_sb[:, :], 1.0)

            # prefetch groups 1-2 + the first beta tiles
            nc.sync.dma_start(pl_tiles[1][:, :], alpha_d[P : 2 * P, :])
            nc.sync.dma_start(pl_tiles[2][:, :], alpha_d[2 * P : 3 * P, :])
            b_tiles = [
                bp.tile([ECH, NCH * P], bf16, tag=f"b{t}", name=f"bt{t}")
                for t in range(T)
            ]
            for t in range(T):
                nc.sync.dma_start(
                    b_tiles[t][:, :], beta_d[:, t * NCH * P : (t + 1) * NCH * P]
                )

            w_sb = pp.tile([P, NG * GF], bf16)    # softmax weights, grouped
            ed_sb = pp.tile([P, T * EDW], f32)    # DP state, zero-init
            tmp_sb = pp.tile([P, T * SW], f32)    # DP step scratch
            nc.vector.memset(ed_sb[:, :], 0.0)

            ed3 = ed_sb.rearrange("p (t k) -> p t k", t=T)
            # softmax weights sum to 1 over zero-depth preds => ed[2] = 1
            nc.vector.memset(ed3[:, :, 2:3], 1.0)
            tmp3 = tmp_sb.rearrange("p (t k) -> p t k", k=SW)

            c_ps = psc.tile([P, T * EDW], f32, tag="c", bufs=1)

            for g in range(NG):
                pl = pl_tiles[g % 3]
                pv = pl.rearrange("p (o f) -> p o f", o=NPL)

                # max tree over 7 planes, all operands dense bf16 (2x mode)
                mxg = trp2.tile([P, GF], bf16, tag="mx")
                if g == 0:
                    # chase the three g0 DMA pieces
                    lb = trp1.tile([P, GF], bf16, tag="lb")
                    nc.vector.tensor_tensor(lb[:, :], pv[:, 0, :], pv[:, 1, :], Alu.max)
                    la = trp1.tile([P, GF], bf16, tag="la")
                    nc.vector.tensor_tensor(la[:, :], pv[:, 2, :], pv[:, 3, :], Alu.max)
                    nc.vector.tensor_tensor(lb[:, :], lb[:, :], la[:, :], Alu.max)
                    lc = trp1.tile([P, GF], bf16, tag="lc")
                    nc.vector.tensor_tensor(lc[:, :], pv[:, 4, :], pv[:, 5, :], Alu.max)
                    nc.vector.tensor_tensor(lc[:, :], lc[:, :], pv[:, 6, :], Alu.max)
                    nc.vector.tensor_tensor(mxg[:, :], lb[:, :], lc[:, :], Alu.max)
                else:
                    # L1: max(planes 0-2, planes 3-5) in one dense op
                    la = trp1.tile([P, 3 * GF], bf16, tag="la")
                    l3 = la.rearrange("p (i f) -> p i f", i=3)
                    nc.vector.tensor_tensor(
                        l3[:, :, :], pv[:, 0:3, :], pv[:, 3:6, :], Alu.max
                    )
                    lb = trp1.tile([P, GF], bf16, tag="lb")
                    nc.vector.tensor_tensor(
                        lb[:, :], l3[:, 0, :], l3[:, 1, :], Alu.max
                    )
                    lc = trp1.tile([P, GF], bf16, tag="lc")
                    nc.vector.tensor_tensor(lc[:, :], l3[:, 2, :], pv[:, 6, :], Alu.max)
                    nc.vector.tensor_tensor(mxg[:, :], lb[:, :], lc[:, :], Alu.max)

                # reuse buffer 0 for group 3 once group 0's tree is done
                if g == 0:
                    nc.sync.dma_start(
                        pl_tiles[0][:, :], alpha_d[3 * P : 4 * P, :]
                    )

                # softmax numerator without max-subtraction (|logits| <~ 6)
                e_sl = w_sb[:, g * GF : (g + 1) * GF]
                nc.scalar.activation(e_sl, mxg[:, :], Act.Exp)

                # two bf16 pair-add levels at 2x, then a 4-wide 1x reduce
                ph = sp.tile([P, T * GN * 8], bf16, tag="ph")
                e4 = e_sl.rearrange("p (n k) -> p n k", k=SW)
                nc.vector.tensor_add(
                    ph.rearrange("p (n k) -> p n k", k=8),
                    e4[:, :, 0:8],
                    e4[:, :, 8:16],
                )
                pq = sp.tile([P, T * GN * 4], bf16, tag="pq")
                p8 = ph.rearrange("p (n k) -> p n k", k=8)
                nc.vector.tensor_add(
                    pq.rearrange("p (n k) -> p n k", k=4),
                    p8[:, :, 0:4],
                    p8[:, :, 4:8],
                )
                s_g = sp.tile([P, T * GN], f32, tag="s")
                nc.vector.reduce_sum(
                    s_g[:, :], pq.rearrange("p (n k) -> p n k", k=4), axis=X.X
                )
                lns = sp.tile([P, T * GN], f32, tag="lns")
                nc.scalar.activation(lns[:, :], s_g[:, :], Act.Ln)
                # fused exp(-ln s) + broadcast-expand to [., n, 16] on ACT so
                # the normalize multiply below runs dense bf16 at 2x
                rse = sp.tile([P, GF], bf16, tag="rse")
                nc.scalar.activation(
                    rse.rearrange("p (n k) -> p n k", k=SW),
                    lns.rearrange("p (n o) -> p n o", o=1).broadcast_to(
                        (P, T * GN, SW)
                    ),
                    Act.Exp,
                    scale=-1.0,
                )
                nc.vector.tensor_mul(e_sl, e_sl, rse[:, :])

                # DP steps for this group's nodes (all 8 stage slots at once)
                wg = w_sb[:, g * GF : (g + 1) * GF].rearrange(
                    "p (t n k) -> p t n k", t=T, k=SW
                )
                if g == 0:
                    # ed[3] = 1 + w[3,2]*ed[2] = 1 + w[3,2] (scalar engine)
                    nc.scalar.add(ed3[:, :, 3:4], wg[:, :, 1, 2:3], 1.0)
                for nl in range(2 if g == 0 else 0, GN):
                    j = g * GN + nl + 2
                    wid = min(j, SW)
                    base = j - wid
                    nc.vector.scalar_tensor_tensor(
                        tmp3[:, :, 0:wid],
                        ed3[:, :, base : base + wid],
                        1.0,
                        wg[:, :, nl, 0:wid],
                        Alu.add,
                        Alu.mult,
                    )
                    nc.vector.reduce_sum(
                        ed3[:, :, j : j + 1], tmp3[:, :, 0:wid], axis=X.X
                    )

            # ---- beta phase (low priority: fills scalar/PE gaps) ----
            for t in range(T):
                eb_t = ep.tile([ECH, NCH * P], bf16, tag="eb")
                half = NCH * P // 2
                nc.scalar.activation(
                    eb_t[:, 0:half], b_tiles[t][:, 0:half], Act.Exp
                )
                nc.scalar.activation(
                    eb_t[:, half:], b_tiles[t][:, half:], Act.Exp
                )
                for c in range(NCH):
                    nc.tensor.matmul(
                        c_ps[:, t * EDW : (t + 1) * EDW],
                        eb_t[:, c * P : (c + 1) * P],
                        mt_sb[:, c * EDW : (c + 1) * EDW],
                        start=(c == 0),
                        stop=(c == NCH - 1),
                    )

            # ---- final dots: batched over all 8 stage slots ----
            prod = fp_.tile([P, T * (EDW - 1)], f32, tag="prod")
            q = fp_.tile([P, T], f32, tag="q")
            cv = c_ps.rearrange("p (t k) -> p t k", k=EDW)
            nc.vector.scalar_tensor_tensor(
                prod.rearrange("p (t k) -> p t k", k=EDW - 1),
                ed3[:, :, 0 : EDW - 1],
                0.0,
                cv[:, :, 0 : EDW - 1],
                Alu.add,
                Alu.mult,
            )
            nc.vector.reduce_sum(
                q.rearrange("p (t k) -> p t k", k=1),
                prod.rearrange("p (t k) -> p t k", k=EDW - 1),
                axis=X.X,
            )
            rsb = fp_.tile([P, T], f32, tag="rsb")
            nc.vector.reciprocal(rsb[:, :], cv[:, :, EDW - 1])
            rst = fp_.tile([P, T], f32, tag="rst")
            nc.vector.tensor_mul(rst[:, :], rsb[:, :], theta_sb[:, :])
            acc = fp_.tile([P, T], f32, tag="acc")
            nc.vector.tensor_mul(acc[:, :], q[:, :], rst[:, :])

            # ---- final reduction: 8 cols then 128 partitions ----
            accsum = fp_.tile([P, 1], f32, tag="accsum")
            nc.vector.reduce_sum(accsum[:, :], acc[:, :], axis=X.X)
            out_ps = psc.tile([1, 1], f32, tag="outp", bufs=1)
            nc.tensor.matmul(
                out_ps[:, :], accsum[:, :], ones_sb[:, :], start=True, stop=True
            )
            out_sb = fp_.tile([1, 1], f32, tag="outs")
            nc.scalar.copy(out_sb[:, :], out_ps[:, :])
            nc.sync.dma_start(out_d[:, :], out_sb[:, :])

    return nc


def _get_compiled():
    if "nc" not in _CACHE:
        _CACHE["nc"] = _build_nc()
        _CACHE["consts"] = _host_consts()
    return _CACHE["nc"], _CACHE["consts"]


def _in_maps(alpha, beta, theta):
    import ml_dtypes

    mt, mask = _get_compiled()[1]
    alpha = np.ascontiguousarray(alpha, dtype=np.float32)
    beta = np.ascontiguousarray(beta, dtype=np.float32)
    theta = np.ascontiguousarray(theta, dtype=np.float32)
    alpha_bf = alpha.astype(ml_dtypes.bfloat16)
    beta_bf = beta.astype(ml_dtypes.bfloat16)
    maps = []
    for c in range(N_CORES):
        sl = slice(c * S_CORE, (c + 1) * S_CORE)
        # [t, p, g, nl, k, o] -> [g, p, o, t, nl, k], drop op 7
        A = alpha_bf[sl].reshape(T, P, NG, GN, SW, 8)
        planes = np.ascontiguousarray(A.transpose(2, 1, 5, 0, 3, 4)[:, :, :NPL])
        # bake the window-validity mask into group 0's padding slots:
        # node n has valid rows k < n+2; exp(-300) underflows to 0
        inv = np.zeros((GN, SW), bool)
        for n in range(NMASK):
            inv[n, n + 2 :] = True
        planes[0][:, :, :, inv] = np.float32(-300.0).astype(planes.dtype)
        # [el, t*2048 + ch*128 + p] = beta[t*128 + p, ch*126 + el]
        beta_t = np.ascontiguousarray(
            beta_bf[sl].reshape(T, P, NCH, ECH).transpose(3, 0, 2, 1).reshape(ECH, -1)
        )
        maps.append(
            {
                "alpha_p": planes.reshape(NG * P, NPL * GF),
                "beta_t": beta_t,
                "theta_t": np.ascontiguousarray(theta[sl].reshape(T, P).T),
                "mt_c": mt,
            }
        )
    return maps


def _run(alpha, beta, theta, **spmd_kwargs):
    from concourse.bass_utils import run_bass_kernel_spmd

    nc, _ = _get_compiled()
    res = run_bass_kernel_spmd(
        nc, _in_maps(alpha, beta, theta), core_ids=list(range(N_CORES)), **spmd_kwargs
    )
    total = np.float32(0.0)
    for r in res.results:
        total += np.float32(r["loss_part"][0, 0])
    return np.float32(total), res


def kernel(alpha, beta, theta):
    out, _ = _run(alpha, beta, theta)
    return out
